# revision 1
# baseline (speedup 1.0000x reference)
"""Trainium2 Bass kernel for nn_GroupAttentionLayer (sparse block attention).

Strategy (8 NeuronCores, SPMD):
  Query sharding: core i handles batch b=i//2, query-pixel half h=i%2
  (2048 query pixels each). Attention, Conv_K accumulator and CBL_Q are
  computed per-batch with channel-major layouts so every reduction lands
  on the natural engine axis:

    scores^T[k,q] = Qc[:,k].T @ Xq[:,q]          (PE, contract channels)
    E = exp(scores/8)                             (ACT, fused 1/8 scale)
    D_bcast = blockmap.T @ E                      (PE; per-64-block sums,
                                                   pre-broadcast over partitions)
    A = E * recip(D_bcast)                        (DVE/POOL split)
    agg^T[c,q] += x_block[k,:].T @ A              (PE, contract keys, PSUM acc,
                                                   Conv_K folded in as first matmul)

  Two collectives: AllReduce of BN_Q batch stats ([128,2]) and AllGather
  of the per-core z1 shard (1 MB/rank). The epilogue (BN1 + spatial
  softmax + CBL_O) runs redundantly on every core from the gathered
  full tensor, so no further syncs are needed.

Host side: shards/transposes inputs with numpy, assembles the full
output from core 0's channel-major result.
"""

import numpy as np

B, H, W, C = 4, 64, 64, 128
RF = 8
EPS = 1e-3
ALPHA = 0.1
N_CORES = 8
HWPIX = H * W            # 4096 pixels per batch
QSH = HWPIX * B // N_CORES  # 2048 query pixels per core
PW = W + 2               # 66, padded row width
PADN = PW * (H + 2)      # 4356 padded columns
NKT = HWPIX // 128       # 32 key tiles per batch
NQT = QSH // 512         # 4 query tiles per core
NCH = (HWPIX * B) // 512  # 32 epilogue chunks
F32 = None               # set on first build (mybir.dt.float32)

# 1 of every DVE_EVERY normalize-multiplies runs on DVE; the rest on POOL
DVE_EVERY = 3

DEBUG = False  # adds intermediate-tensor outputs for bisection

_CACHE = {}


def _build_program():
    import concourse.bacc as bacc
    import concourse.tile as tile
    from concourse import mybir

    f32 = mybir.dt.float32
    f32r = mybir.dt.float32r
    AF = mybir.ActivationFunctionType
    OP = mybir.AluOpType
    AX = mybir.AxisListType

    nc = bacc.Bacc("TRN2", target_bir_lowering=False, debug=False,
                   enable_asserts=True, num_devices=N_CORES)

    # per-core inputs
    d_xb = nc.dram_tensor("xb", [HWPIX, C], f32, kind="ExternalInput").ap()
    d_xqT = nc.dram_tensor("xqT", [C, QSH], f32, kind="ExternalInput").ap()
    d_xpadT = nc.dram_tensor("xpadT", [C, PADN], f32, kind="ExternalInput").ap()
    # shared inputs
    d_wq9 = nc.dram_tensor("wq9", [9, C, C], f32, kind="ExternalInput").ap()
    d_wk = nc.dram_tensor("wk", [C, C], f32, kind="ExternalInput").ap()
    d_wo = nc.dram_tensor("wo", [C, C], f32, kind="ExternalInput").ap()
    d_vecs = nc.dram_tensor("vecs", [6, C], f32, kind="ExternalInput").ap()
    d_bm = nc.dram_tensor("bm", [C, C], f32, kind="ExternalInput").ap()
    # output: full channel-major result (identical on every core)
    d_outT = nc.dram_tensor("outT", [C, B * HWPIX], f32, kind="ExternalOutput").ap()
    if DEBUG:
        d_dbg_qc = nc.dram_tensor("dbg_qc", [C, HWPIX], f32,
                                  kind="ExternalOutput").ap()
        d_dbg_z1 = nc.dram_tensor("dbg_z1", [C, QSH], f32,
                                  kind="ExternalOutput").ap()
        d_dbg_zfull = nc.dram_tensor("dbg_zfull", [C, B * HWPIX], f32,
                                     kind="ExternalOutput").ap()

    with tile.TileContext(nc) as tc:
        with tc.tile_pool(name="const", bufs=1) as const, \
             tc.tile_pool(name="big", bufs=1) as big, \
             tc.tile_pool(name="work", bufs=6) as work, \
             tc.tile_pool(name="tmp2", bufs=2) as tmp2p, \
             tc.tile_pool(name="zbig", bufs=1) as zbig, \
             tc.tile_pool(name="small", bufs=2) as small, \
             tc.tile_pool(name="ps", bufs=3, space="PSUM") as ps, \
             tc.tile_pool(name="psA", bufs=2, space="PSUM") as psA, \
             tc.tile_pool(name="dram", bufs=1, space="DRAM") as dram:

            # ---------------- loads ----------------
            Xpad = big.tile([C, PADN], f32r)
            nc.sync.dma_start(Xpad[:], d_xpadT[:].bitcast(f32r))
            Xq = big.tile([C, QSH], f32r)
            nc.sync.dma_start(Xq[:], d_xqT[:].bitcast(f32r))
            Xnat = big.tile([128, NKT, C], f32r)
            nc.scalar.dma_start(
                Xnat[:], d_xb.rearrange("(t p) c -> p t c", p=128).bitcast(f32r))
            Wq_s = const.tile([C, 9, C], f32r)
            nc.scalar.dma_start(
                Wq_s[:], d_wq9.rearrange("t ci co -> ci t co").bitcast(f32r))
            Wk_s = const.tile([C, C], f32r)
            nc.sync.dma_start(Wk_s[:], d_wk[:].bitcast(f32r))
            Wo_s = const.tile([C, C], f32r)
            nc.sync.dma_start(Wo_s[:], d_wo[:].bitcast(f32r))
            V = const.tile([C, 6], f32)
            nc.scalar.dma_start(V[:], d_vecs.rearrange("v c -> c v"))
            Bb = const.tile([C, C], f32r)
            nc.sync.dma_start(Bb[:], d_bm[:].bitcast(f32r))
            eps_t = const.tile([C, 1], f32)
            nc.vector.memset(eps_t[:], EPS)

            Xpv = Xpad[:].rearrange("p (r c) -> p r c", r=H + 2)

            # ---------------- CBL_Q: conv3x3 + batch stats ----------------
            # Conv output is produced directly in BLOCK-MAJOR key order:
            # chunk t covers block-row n=t, column order (m, p, q) so that
            # Qc column n*512 + m*64 + p*8 + q is pixel (8n+p, 8m+q). Each
            # 128-column slice of Qc is then two complete 8x8 blocks,
            # matching the blockmap and the host-side permutation of xb.
            Zq = zbig.tile([C, 8, 512], f32, tag="zbig",
                           padded_shape=[C, 8, B * HWPIX // 8])  # shares Z slot
            qstats = small.tile([C, 8, 6], f32)
            for t in range(8):
                pq = psA.tile([C, 512], f32, tag="agg")
                for tap in range(9):
                    dh, dw = tap // 3 - 1, tap % 3 - 1
                    rhs = Xpv[:, t * 8 + 1 + dh: t * 8 + 9 + dh,
                              1 + dw: 65 + dw].rearrange(
                                  "c p (m q) -> c m p q", m=8)
                    nc.tensor.matmul(pq[:], Wq_s[:, tap, :], rhs,
                                     start=(tap == 0), stop=(tap == 8))
                nc.vector.bn_stats(qstats[:, t, :], pq[:])
                nc.scalar.copy(Zq[:, t, :], pq[:])

            qmv = small.tile([C, 2], f32)
            nc.vector.bn_aggr(qmv[:], qstats[:])
            # partial sums for the global (cross-core) stats:
            #   sums[:,0] = mean * 4096 ; sums[:,1] = (var + mean^2) * 4096
            sums = small.tile([C, 2], f32)
            nc.vector.tensor_scalar_mul(sums[:, 0:1], qmv[:, 0:1], float(HWPIX))
            m2 = small.tile([C, 1], f32)
            nc.vector.tensor_mul(m2[:], qmv[:, 0:1], qmv[:, 0:1])
            nc.vector.tensor_add(m2[:], m2[:], qmv[:, 1:2])
            nc.vector.tensor_scalar_mul(sums[:, 1:2], m2[:], float(HWPIX))

            st_in = dram.tile([C, 2], f32)
            st_out = dram.tile([C, 2], f32, addr_space="Shared")
            nc.sync.dma_start(st_in[:], sums[:])
            nc.gpsimd.collective_compute(
                "AllReduce", mybir.AluOpType.add,
                replica_groups=[list(range(N_CORES))],
                ins=[st_in.opt()], outs=[st_out.opt()])
            gst = small.tile([C, 2], f32)
            nc.sync.dma_start(gst[:], st_out[:])

            # global mean / var (each batch appears twice in the sum)
            TOT = float(HWPIX * N_CORES)
            mean_g = small.tile([C, 1], f32)
            nc.vector.tensor_scalar_mul(mean_g[:], gst[:, 0:1], 1.0 / TOT)
            negvar = small.tile([C, 1], f32)
            # (mean*mean) - E[z^2]  ->  -var
            ez2 = small.tile([C, 1], f32)
            nc.vector.tensor_scalar_mul(ez2[:], gst[:, 1:2], 1.0 / TOT)
            nc.vector.scalar_tensor_tensor(negvar[:], mean_g[:], mean_g[:],
                                           ez2[:], op0=OP.mult,
                                           op1=OP.subtract)
            std = small.tile([C, 1], f32)
            nc.scalar.activation(std[:], negvar[:], AF.Sqrt,
                                 scale=-1.0, bias=eps_t[:])
            rstd = small.tile([C, 1], f32)
            nc.vector.reciprocal(rstd[:], std[:])
            aq = small.tile([C, 1], f32)
            nc.vector.tensor_mul(aq[:], rstd[:], V[:, 0:1])
            bq = small.tile([C, 1], f32)
            nc.vector.tensor_scalar(bq[:], mean_g[:], aq[:], -1.0,
                                    op0=OP.mult, op1=OP.mult)
            nc.vector.tensor_add(bq[:], bq[:], V[:, 1:2])

            # q = leaky(aq*z + bq); Zq is already block-major
            Qc = big.tile([C, HWPIX], f32r)
            Qv = Qc[:].rearrange("p (t f) -> p t f", f=512)
            for t in range(8):
                tmp = tmp2p.tile([C, 512], f32, tag="tmp2")
                nc.scalar.activation(tmp[:], Zq[:, t, :], AF.Identity,
                                     scale=aq[:], bias=bq[:])
                nc.vector.scalar_tensor_tensor(Qv[:, t, :], tmp[:], ALPHA,
                                               tmp[:], op0=OP.mult,
                                               op1=OP.max)

            # ---------------- attention main loop ----------------
            z1s = big.tile([C, NQT, 512], f32)
            AGW = QSH + 8
            ag_in = dram.tile([C, AGW], f32)
            ag_out = dram.tile([N_CORES * C, AGW], f32, addr_space="Shared")
            qs1 = small.tile([C, NQT, 6], f32)
            for qt in range(NQT):
                pagg = psA.tile([C, 512], f32, tag="agg")
                # Conv_K accumulator folded into the attention PSUM group
                nc.tensor.matmul(pagg[:], Wk_s[:],
                                 Xq[:, qt * 512:(qt + 1) * 512],
                                 start=True, stop=False)
                for kt in range(NKT):
                    psS = ps.tile([C, 512], f32, tag="s")
                    nc.tensor.matmul(psS[:], Qc[:, kt * 128:(kt + 1) * 128],
                                     Xq[:, qt * 512:(qt + 1) * 512],
                                     start=True, stop=True)
                    E = work.tile([C, 512], f32r, tag="E")
                    nc.scalar.activation(E[:], psS[:], AF.Exp, scale=1.0 / RF)
                    psD = ps.tile([C, 512], f32, tag="d")
                    nc.tensor.matmul(psD[:], Bb[:], E[:],
                                     start=True, stop=True)
                    R = work.tile([C, 512], f32, tag="R")
                    nc.vector.reciprocal_approx_fast(R[:], psD[:])
                    A = work.tile([C, 512], f32r, tag="A")
                    if kt % DVE_EVERY == DVE_EVERY - 1:
                        nc.vector.tensor_mul(A[:], E[:], R[:])
                    else:
                        nc.gpsimd.tensor_mul(A[:], E[:], R[:])
                    nc.tensor.matmul(pagg[:], Xnat[:, kt, :], A[:],
                                     start=False, stop=(kt == NKT - 1))
                nc.scalar.copy(z1s[:, qt, :], pagg[:])
                nc.vector.bn_stats(qs1[:, qt, :], pagg[:])
                nc.sync.dma_start(ag_in[:, qt * 512:(qt + 1) * 512],
                                  z1s[:, qt, :])

            sh_mv = small.tile([C, 2], f32)
            nc.vector.bn_aggr(sh_mv[:], qs1[:])
            sh_sums = small.tile([C, 2], f32)
            nc.vector.tensor_scalar_mul(sh_sums[:, 0:1], sh_mv[:, 0:1],
                                        float(QSH))
            shm2 = small.tile([C, 1], f32)
            nc.vector.tensor_mul(shm2[:], sh_mv[:, 0:1], sh_mv[:, 0:1])
            nc.vector.tensor_add(shm2[:], shm2[:], sh_mv[:, 1:2])
            nc.vector.tensor_scalar_mul(sh_sums[:, 1:2], shm2[:], float(QSH))
            nc.sync.dma_start(ag_in[:, QSH:QSH + 2], sh_sums[:])

            if DEBUG:
                nc.sync.dma_start(d_dbg_qc[:], Qc[:])
                nc.sync.dma_start(d_dbg_z1[:],
                                  z1s[:].rearrange("c a b -> c (a b)"))

            nc.gpsimd.collective_compute(
                "AllGather", mybir.AluOpType.bypass,
                replica_groups=[list(range(N_CORES))],
                ins=[ag_in.opt()], outs=[ag_out.opt()])

            # ---------------- epilogue (redundant on all cores) ------------
            Z = zbig.tile([C, B * HWPIX], f32, tag="zbig")
            ag_v = ag_out[:].rearrange("(r p) f -> p r f", r=N_CORES)
            Zr16 = Z[:].rearrange("p (r h f) -> p r h f", r=N_CORES, h=2)
            for r in range(N_CORES):
                for hh in range(2):
                    eng = nc.sync if (2 * r + hh) % 2 == 0 else nc.scalar
                    eng.dma_start(Zr16[:, r, hh, :],
                                  ag_v[:, r, hh * (QSH // 2):(hh + 1) * (QSH // 2)])
            Zv = Z[:].rearrange("p (t f) -> p t f", f=512)
            st8 = small.tile([C, N_CORES, 2], f32)
            nc.sync.dma_start(st8[:], ag_v[:, :, QSH:QSH + 2])
            if DEBUG:
                nc.sync.dma_start(d_dbg_zfull[:], Z[:])

            TOT1 = float(B * HWPIX)
            gsum1 = small.tile([C, 2], f32)
            nc.vector.tensor_reduce(
                gsum1[:], st8[:].rearrange("c r j -> c j r"),
                axis=AX.X, op=OP.add)
            mean1 = small.tile([C, 1], f32)
            nc.vector.tensor_scalar_mul(mean1[:], gsum1[:, 0:1], 1.0 / TOT1)
            ez21 = small.tile([C, 1], f32)
            nc.vector.tensor_scalar_mul(ez21[:], gsum1[:, 1:2], 1.0 / TOT1)
            negv1 = small.tile([C, 1], f32)
            nc.vector.scalar_tensor_tensor(negv1[:], mean1[:], mean1[:],
                                           ez21[:], op0=OP.mult,
                                           op1=OP.subtract)
            std1 = small.tile([C, 1], f32)
            nc.scalar.activation(std1[:], negv1[:], AF.Sqrt,
                                 scale=-1.0, bias=eps_t[:])
            rstd1 = small.tile([C, 1], f32)
            nc.vector.reciprocal(rstd1[:], std1[:])
            a1 = small.tile([C, 1], f32)
            nc.vector.tensor_mul(a1[:], rstd1[:], V[:, 2:3])
            b1 = small.tile([C, 1], f32)
            nc.vector.tensor_scalar(b1[:], mean1[:], a1[:], -1.0,
                                    op0=OP.mult, op1=OP.mult)
            nc.vector.tensor_add(b1[:], b1[:], V[:, 3:4])

            # exp(BN1(z)) in place, with per-chunk sums from the ACT pass
            esums = small.tile([C, NCH], f32)
            for t in range(NCH):
                nc.scalar.activation(Zv[:, t, :], Zv[:, t, :], AF.Exp,
                                     scale=a1[:], bias=b1[:],
                                     accum_out=esums[:, t:t + 1])
            rb = small.tile([C, B], f32)
            for b in range(B):
                bsum = small.tile([C, 1], f32, tag="bsum")
                nc.vector.tensor_reduce(
                    bsum[:], esums[:, b * 8:(b + 1) * 8],
                    axis=AX.X, op=OP.add)
                nc.vector.reciprocal(rb[:, b:b + 1], bsum[:])
            # CBL_O: softmax-normalize each chunk into a small fp32r
            # staging tile, 1x1 conv + batch stats, overwrite Z with the
            # conv output (the exp values are no longer needed).
            stO = big.tile([C, NCH, 6], f32)
            for t in range(NCH):
                Yt = tmp2p.tile([C, 512], f32r, tag="Yt")
                nc.vector.tensor_scalar_mul(Yt[:], Zv[:, t, :],
                                            rb[:, t // 8:t // 8 + 1])
                pzo = ps.tile([C, 512], f32, tag="s")
                nc.tensor.matmul(pzo[:], Wo_s[:], Yt[:],
                                 start=True, stop=True)
                nc.vector.bn_stats(stO[:, t, :], pzo[:])
                nc.scalar.copy(Zv[:, t, :], pzo[:])
            mvO = small.tile([C, 2], f32)
            nc.vector.bn_aggr(mvO[:], stO[:])
            stdO = small.tile([C, 1], f32)
            nc.scalar.activation(stdO[:], mvO[:, 1:2], AF.Sqrt,
                                 scale=1.0, bias=eps_t[:])
            rstdO = small.tile([C, 1], f32)
            nc.vector.reciprocal(rstdO[:], stdO[:])
            aO = small.tile([C, 1], f32)
            nc.vector.tensor_mul(aO[:], rstdO[:], V[:, 4:5])
            bO = small.tile([C, 1], f32)
            nc.vector.tensor_scalar(bO[:], mvO[:, 0:1], aO[:], -1.0,
                                    op0=OP.mult, op1=OP.mult)
            nc.vector.tensor_add(bO[:], bO[:], V[:, 5:6])

            for t in range(NCH):
                tmp = tmp2p.tile([C, 512], f32, tag="tmp2")
                nc.scalar.activation(tmp[:], Zv[:, t, :], AF.Identity,
                                     scale=aO[:], bias=bO[:])
                nc.vector.scalar_tensor_tensor(Zv[:, t, :], tmp[:], ALPHA,
                                               tmp[:], op0=OP.mult,
                                               op1=OP.max)
                eng = nc.sync if t % 2 == 0 else nc.scalar
                eng.dma_start(d_outT[:, t * 512:(t + 1) * 512],
                              Zv[:, t, :])

    nc.compile()
    return nc


def _get_runner():
    if "runner" in _CACHE:
        return _CACHE["runner"]
    import jax
    import numpy as np
    from jax.sharding import Mesh, PartitionSpec
    from jax.experimental.shard_map import shard_map
    from concourse import mybir
    from concourse.bass2jax import (_bass_exec_p, install_neuronx_cc_hook,
                                    partition_id_tensor)

    nc = _build_program()
    install_neuronx_cc_hook()

    in_names, out_names, out_avals, zero_outs = [], [], [], []
    partition_name = nc.partition_id_tensor.name if nc.partition_id_tensor else None
    for alloc in nc.m.functions[0].allocations:
        if not isinstance(alloc, mybir.MemoryLocationSet):
            continue
        name = alloc.memorylocations[0].name
        if alloc.kind == "ExternalInput":
            if name != partition_name:
                in_names.append(name)
        elif alloc.kind == "ExternalOutput":
            shape = tuple(alloc.tensor_shape)
            dtype = mybir.dt.np(alloc.dtype)
            out_names.append(name)
            out_avals.append(jax.core.ShapedArray(shape, dtype))
            zero_outs.append(np.zeros(shape, dtype))
    n_params = len(in_names)
    n_outs = len(out_avals)
    all_in_names = list(in_names) + list(out_names)
    if partition_name is not None:
        all_in_names.append(partition_name)

    def _body(*args):
        operands = list(args)
        if partition_name is not None:
            operands.append(partition_id_tensor())
        outs = _bass_exec_p.bind(
            *operands,
            out_avals=tuple(out_avals),
            in_names=tuple(all_in_names),
            out_names=tuple(out_names),
            lowering_input_output_aliases=(),
            sim_require_finite=True,
            sim_require_nnan=True,
            nc=nc,
        )
        return tuple(outs)

    donate = tuple(range(n_params, n_params + n_outs))
    try:
        devices = jax.devices("axon")[:N_CORES]
    except RuntimeError:
        devices = jax.devices()[:N_CORES]
    mesh = Mesh(np.asarray(devices), ("core",))
    in_specs = (PartitionSpec("core"),) * (n_params + n_outs)
    out_specs = (PartitionSpec("core"),) * n_outs
    sharded = jax.jit(
        shard_map(_body, mesh=mesh, in_specs=in_specs, out_specs=out_specs,
                  check_rep=False),
        donate_argnums=donate, keep_unused=True)

    def run(in_maps):
        per_core = [[np.asarray(m[name]) for name in in_names] for m in in_maps]
        concat_in = [np.concatenate([per_core[c][i] for c in range(N_CORES)],
                                    axis=0) for i in range(n_params)]
        concat_zeros = [np.zeros((N_CORES * z.shape[0], *z.shape[1:]), z.dtype)
                        for z in zero_outs]
        out_arrs = jax.block_until_ready(sharded(*concat_in, *concat_zeros))
        return [
            {name: np.asarray(out_arrs[i]).reshape(N_CORES, *out_avals[i].shape)[c]
             for i, name in enumerate(out_names)}
            for c in range(N_CORES)
        ]

    _CACHE["runner"] = run
    return run


def _make_blockmap():
    bm = np.zeros((C, C), np.float32)
    idx = np.arange(C)
    bm[(idx[:, None] // 64) == (idx[None, :] // 64)] = 1.0
    return bm


def kernel(x, Wq, bq, gq, btq, Wk, bk, g1, bt1, Wo, bo, go, bto):
    """Full inputs -> full output. Conv biases cancel inside training-mode
    BN (the mean subtraction removes any per-channel constant), so bq/bk/bo
    never enter the device program."""
    x = np.asarray(x, np.float32)
    run = _get_runner()

    wq9 = np.ascontiguousarray(
        np.asarray(Wq, np.float32).reshape(9, C, C))
    wk = np.ascontiguousarray(np.asarray(Wk, np.float32).reshape(C, C))
    wo = np.ascontiguousarray(np.asarray(Wo, np.float32).reshape(C, C))
    vecs = np.ascontiguousarray(np.stack([
        np.asarray(v, np.float32) for v in (gq, btq, g1, bt1, go, bto)]))
    bm = _make_blockmap()

    # block-major key permutation: tile kt=(n,j) holds blocks (n,2j),(n,2j+1)
    # with partition index mb*64 + p*8 + q  (see QcB view in _build_program)
    perm = np.arange(HWPIX).reshape(8, 8, 8, 8).transpose(0, 2, 1, 3).reshape(-1)

    in_maps = []
    for core in range(N_CORES):
        b, h = core // 2, core % 2
        xb = np.ascontiguousarray(x[b].reshape(HWPIX, C))
        xbT = xb.T  # [C, HWPIX]
        xqT = np.ascontiguousarray(xbT[:, h * QSH:(h + 1) * QSH])
        xpadT = np.zeros((C, H + 2, W + 2), np.float32)
        xpadT[:, 1:H + 1, 1:W + 1] = xbT.reshape(C, H, W)
        in_maps.append({
            "xb": np.ascontiguousarray(xb[perm]),
            "xqT": xqT,
            "xpadT": np.ascontiguousarray(xpadT.reshape(C, PADN)),
            "wq9": wq9, "wk": wk, "wo": wo, "vecs": vecs, "bm": bm,
        })

    res = run(in_maps)
    outT = res[0]["outT"]  # [C, B*HWPIX], identical on every core
    return np.ascontiguousarray(outT.T).reshape(B, H, W, C)



# revision 5
# speedup vs baseline: 2.0961x; 2.0961x over previous
"""Trainium2 Bass kernel for nn_GroupAttentionLayer (sparse block attention).

Strategy (8 NeuronCores, SPMD): core i handles batch b=i//2, query-pixel
half h=i%2 (2048 query pixels each). All heavy tensors are bf16 (PSUM
accumulation stays fp32), attention tiles are 1024 queries wide, and the
epilogue is fully SHARDED (the 1x1 conv + BN/softmax reductions are
pointwise, so no core ever needs the full tensor):

    scores^T[k,q] = Qc[:,k].T @ Xq[:,q]      (PE, contract channels, bf16)
    E = exp(scores/8)                        (ACT, bf16 out)
    D_bcast = blockmap.T @ E                 (PE, per-64-block sums)
    A = E / D_bcast                          (DVE tensor_tensor divide,
                                              every POOL_EVERY-th on Pool)
    agg^T[c,q] += x_blk[k,:].T @ A           (PE, PSUM accum, Conv_K folded)

The PE instruction stream is software-pipelined (scores issued 2 iters
ahead, blockmap 1 ahead) so the in-order engine queues never stall on
cross-engine dependencies.

Four tiny AllGathers (cheaper than AllReduce under the collective cost
model) carry the cross-core reductions: BN_Q batch stats, BN_1 batch
stats, per-batch spatial-softmax exp sums, BN_O batch stats. BN shifts
that feed the spatial softmax cancel algebraically (softmax is
shift-invariant) and conv biases cancel inside training-mode BN, so
neither is computed. A per-core one-hot mask input selects this core's
batch pair out of the gathered exp sums.

Host side: shards/transposes/bf16-converts inputs with numpy, assembles
the output from the 8 per-core channel-major shards.
"""

import numpy as np

B, H, W, C = 4, 64, 64, 128
RF = 8
EPS = 1e-3
ALPHA = 0.1
N_CORES = 8
HWPIX = H * W             # 4096 pixels per batch
QSH = HWPIX * B // N_CORES  # 2048 query pixels per core
PW = W + 2                # 66, padded row width
PADN = PW * (H + 2)       # 4356 padded columns
NKT = HWPIX // 128        # 32 key tiles per batch
TW = 1024                 # attention tile width (queries)
NQT = QSH // TW           # 2 query tiles per core
NCC = HWPIX // TW         # 4 conv chunks (2 block-rows each)
NIT = NQT * NKT           # 64 inner iterations

# every POOL_EVERY-th divide runs on Pool (gpsimd), the rest on DVE
POOL_EVERY = 5

_CACHE = {}


def _build_program():
    import concourse.bacc as bacc
    import concourse.tile as tile
    from concourse import mybir

    f32 = mybir.dt.float32
    bf16 = mybir.dt.bfloat16
    AF = mybir.ActivationFunctionType
    OP = mybir.AluOpType
    AX = mybir.AxisListType

    nc = bacc.Bacc("TRN2", target_bir_lowering=False, debug=False,
                   enable_asserts=True, num_devices=N_CORES)

    # per-core inputs (bf16 heavy data, fp32 vectors)
    d_xnat = nc.dram_tensor("xnat", [HWPIX, C], bf16, kind="ExternalInput").ap()
    d_xqT = nc.dram_tensor("xqT", [C, QSH], bf16, kind="ExternalInput").ap()
    d_xpadT = nc.dram_tensor("xpadT", [C, PADN], bf16, kind="ExternalInput").ap()
    d_wq9 = nc.dram_tensor("wq9", [9, C, C], bf16, kind="ExternalInput").ap()
    d_wk = nc.dram_tensor("wk", [C, C], bf16, kind="ExternalInput").ap()
    d_wo = nc.dram_tensor("wo", [C, C], bf16, kind="ExternalInput").ap()
    d_vecs = nc.dram_tensor("vecs", [6, C], f32, kind="ExternalInput").ap()
    d_msk = nc.dram_tensor("msk", [8, C], f32, kind="ExternalInput").ap()
    d_bm = nc.dram_tensor("bm", [C, C], bf16, kind="ExternalInput").ap()
    # output: this core's channel-major shard
    d_outT = nc.dram_tensor("outT", [C, QSH], f32, kind="ExternalOutput").ap()

    with tile.TileContext(nc) as tc:
        with tc.tile_pool(name="const", bufs=1) as const, \
             tc.tile_pool(name="big", bufs=1) as big, \
             tc.tile_pool(name="work", bufs=3) as work, \
             tc.tile_pool(name="tmp2", bufs=2) as tmp2p, \
             tc.tile_pool(name="small", bufs=2) as small, \
             tc.tile_pool(name="ps", bufs=3, space="PSUM") as ps, \
             tc.tile_pool(name="psA", bufs=1, space="PSUM") as psA, \
             tc.tile_pool(name="dram", bufs=1, space="DRAM") as dram:

            # ---------------- loads ----------------
            Xpad = big.tile([C, PADN], bf16)
            nc.sync.dma_start(Xpad[:], d_xpadT[:])
            Xq = big.tile([C, QSH], bf16)
            nc.sync.dma_start(Xq[:], d_xqT[:])
            Xnat = big.tile([128, NKT, C], bf16)
            nc.scalar.dma_start(
                Xnat[:], d_xnat.rearrange("(t p) c -> p t c", p=128))
            Wq_s = const.tile([C, 9, C], bf16)
            nc.scalar.dma_start(
                Wq_s[:], d_wq9.rearrange("t ci co -> ci t co"))
            Wk_s = const.tile([C, C], bf16)
            nc.sync.dma_start(Wk_s[:], d_wk[:])
            Wo_s = const.tile([C, C], bf16)
            nc.sync.dma_start(Wo_s[:], d_wo[:])
            V = const.tile([C, 6], f32)
            nc.scalar.dma_start(V[:], d_vecs.rearrange("v c -> c v"))
            Msk = const.tile([C, 8], f32)
            nc.scalar.dma_start(Msk[:], d_msk.rearrange("v c -> c v"))
            Bb = const.tile([C, C], bf16)
            nc.sync.dma_start(Bb[:], d_bm[:])
            eps_t = const.tile([C, 1], f32)
            nc.vector.memset(eps_t[:], EPS)

            Xpv = Xpad[:].rearrange("p (r c) -> p r c", r=H + 2)

            # ---------------- CBL_Q: conv3x3 + batch stats ----------------
            # Conv output in BLOCK-MAJOR key order: chunk t covers block rows
            # n=2t,2t+1; column n*512 + m*64 + p*8 + q is pixel (8n+p, 8m+q).
            Zq = big.tile([C, NCC, TW], bf16)
            qstats = small.tile([C, 2 * NCC, 6], f32)
            for t in range(NCC):
                pq = ps.tile([C, TW], f32, tag="ps")
                for tap in range(9):
                    dh, dw = tap // 3 - 1, tap % 3 - 1
                    rhs = Xpv[:, t * 16 + 1 + dh: t * 16 + 17 + dh,
                              1 + dw: 65 + dw].rearrange(
                                  "c (n p) (m q) -> c n m p q", n=2, m=8)
                    nc.tensor.matmul(pq[:], Wq_s[:, tap, :], rhs,
                                     start=(tap == 0), stop=(tap == 8))
                nc.vector.bn_stats(qstats[:, 2 * t, :], pq[:, :512])
                nc.vector.bn_stats(qstats[:, 2 * t + 1, :], pq[:, 512:])
                nc.gpsimd.tensor_copy(Zq[:, t, :], pq[:])

            qmv = small.tile([C, 2], f32)
            nc.vector.bn_aggr(qmv[:], qstats[:])
            # partial sums for the cross-core stats:
            #   sums[:,0] = mean * 4096 ; sums[:,1] = (var + mean^2) * 4096
            sums = small.tile([C, 2], f32)
            nc.vector.tensor_scalar_mul(sums[:, 0:1], qmv[:, 0:1], float(HWPIX))
            m2 = small.tile([C, 1], f32)
            nc.vector.tensor_mul(m2[:], qmv[:, 0:1], qmv[:, 0:1])
            nc.vector.tensor_add(m2[:], m2[:], qmv[:, 1:2])
            nc.vector.tensor_scalar_mul(sums[:, 1:2], m2[:], float(HWPIX))

            grp = [list(range(N_CORES))]

            def gather8(name, src, width):
                """AllGather a [C,width] fp32 tile -> [C,8,width] on-chip."""
                t_in = dram.tile([C, width], f32, tag=f"{name}_in")
                t_out = dram.tile([N_CORES * C, width], f32,
                                  addr_space="Shared", tag=f"{name}_out")
                nc.sync.dma_start(t_in[:], src[:])
                nc.gpsimd.collective_compute(
                    "AllGather", OP.bypass, replica_groups=grp,
                    ins=[t_in.opt()], outs=[t_out.opt()])
                g = small.tile([C, 8, width], f32, tag=f"{name}_g")
                nc.sync.dma_start(
                    g[:], t_out[:].rearrange("(r p) s -> p r s", r=N_CORES))
                return g

            def bn_affine(gst8, tot, gamma, beta):
                """gst8: [C,8,2] gathered (sum, sumsq) partials. Returns
                (a, b) with a = gamma*rsqrt(var+eps), b = beta - a*mean."""
                gsum = small.tile([C, 2], f32, tag="gsum")
                nc.vector.tensor_reduce(
                    gsum[:], gst8[:].rearrange("c r s -> c s r"),
                    axis=AX.X, op=OP.add)
                mean = small.tile([C, 1], f32, tag="mean")
                nc.vector.tensor_scalar_mul(mean[:], gsum[:, 0:1], 1.0 / tot)
                ez2 = small.tile([C, 1], f32, tag="ez2")
                nc.vector.tensor_scalar_mul(ez2[:], gsum[:, 1:2], 1.0 / tot)
                negvar = small.tile([C, 1], f32, tag="negvar")
                nc.vector.scalar_tensor_tensor(negvar[:], mean[:], mean[:],
                                               ez2[:], op0=OP.mult,
                                               op1=OP.subtract)
                std = small.tile([C, 1], f32, tag="std")
                nc.scalar.activation(std[:], negvar[:], AF.Sqrt,
                                     scale=-1.0, bias=eps_t[:])
                rstd = small.tile([C, 1], f32, tag="rstd")
                nc.vector.reciprocal(rstd[:], std[:])
                a = small.tile([C, 1], f32, tag="acoef")
                nc.vector.tensor_mul(a[:], rstd[:], gamma)
                b = small.tile([C, 1], f32, tag="bcoef")
                nc.vector.tensor_scalar(b[:], mean[:], a[:], -1.0,
                                        op0=OP.mult, op1=OP.mult)
                nc.vector.tensor_add(b[:], b[:], beta)
                return a, b

            gstq = gather8("stq", sums, 2)
            aq, bq = bn_affine(gstq, float(HWPIX * N_CORES),
                               V[:, 0:1], V[:, 1:2])

            # q = leaky(aq*z + bq); Zq is already block-major
            Qc = big.tile([C, HWPIX], bf16)
            Qv = Qc[:].rearrange("p (t f) -> p t f", f=TW)
            for t in range(NCC):
                tmp = tmp2p.tile([C, TW], f32, tag="tmp2")
                nc.scalar.activation(tmp[:], Zq[:, t, :], AF.Identity,
                                     scale=aq[:], bias=bq[:])
                nc.vector.scalar_tensor_tensor(Qv[:, t, :], tmp[:], ALPHA,
                                               tmp[:], op0=OP.mult,
                                               op1=OP.max)

            # ---------------- attention main loop (software-pipelined) ----
            z1 = big.tile([C, NQT, TW], bf16)
            qs1 = small.tile([C, 2 * NQT, 6], f32)
            paggs = [None] * NQT
            Es = [None] * (NIT + 2)
            with nc.allow_low_precision(reason="attn weights in bf16"):
                for i in range(NIT + 2):
                    qt, kt = i // NKT, i % NKT
                    # stage 0: scores (+ per-qt Conv_K group start)
                    if i < NIT:
                        if kt == 0 and qt == 0:
                            pagg0 = psA.tile([C, TW], f32, tag="agg")
                            paggs[0] = pagg0
                            nc.tensor.matmul(pagg0[:], Wk_s[:],
                                             Xq[:, :TW],
                                             start=True, stop=False)
                        if kt == 2 and qt + 1 < NQT:
                            # issued just after agg[qt,31] so the single
                            # psA slot never deadlocks the in-order PE queue
                            pagg_n = psA.tile([C, TW], f32, tag="agg")
                            paggs[qt + 1] = pagg_n
                            nc.tensor.matmul(
                                pagg_n[:], Wk_s[:],
                                Xq[:, (qt + 1) * TW:(qt + 2) * TW],
                                start=True, stop=False)
                        psS = ps.tile([C, TW], f32, tag="ps")
                        nc.tensor.matmul(psS[:],
                                         Qc[:, kt * 128:(kt + 1) * 128],
                                         Xq[:, qt * TW:(qt + 1) * TW],
                                         start=True, stop=True)
                        Ei = work.tile([C, TW], bf16, tag="E")
                        Es[i] = Ei
                        nc.scalar.activation(Ei[:], psS[:], AF.Exp,
                                             scale=1.0 / RF)
                    # stage 1: block-sum matmul + divide (A = E/D in place)
                    if 1 <= i < NIT + 1:
                        j = i - 1
                        psD = ps.tile([C, TW], f32, tag="ps")
                        nc.tensor.matmul(psD[:], Bb[:], Es[j][:],
                                         start=True, stop=True)
                        eng = nc.gpsimd if j % POOL_EVERY == POOL_EVERY - 1 \
                            else nc.vector
                        eng.tensor_tensor(Es[j][:], Es[j][:], psD[:],
                                          op=OP.divide)
                    # stage 2: weighted-sum accumulate
                    if i >= 2:
                        j = i - 2
                        jqt, jkt = j // NKT, j % NKT
                        nc.tensor.matmul(paggs[jqt][:], Xnat[:, jkt, :],
                                         Es[j][:], start=False,
                                         stop=(jkt == NKT - 1))
                        Es[j] = None
                        if jkt == NKT - 1:
                            nc.vector.bn_stats(qs1[:, 2 * jqt, :],
                                               paggs[jqt][:, :512])
                            nc.vector.bn_stats(qs1[:, 2 * jqt + 1, :],
                                               paggs[jqt][:, 512:])
                            nc.gpsimd.tensor_copy(z1[:, jqt, :],
                                                  paggs[jqt][:])
                            paggs[jqt] = None

            # ---------------- epilogue (sharded) ----------------
            # partial BN_1 sums for this core's 2048 pixels
            mv1 = small.tile([C, 2], f32)
            nc.vector.bn_aggr(mv1[:], qs1[:])
            sums1 = small.tile([C, 2], f32)
            nc.vector.tensor_scalar_mul(sums1[:, 0:1], mv1[:, 0:1], float(QSH))
            m21 = small.tile([C, 1], f32)
            nc.vector.tensor_mul(m21[:], mv1[:, 0:1], mv1[:, 0:1])
            nc.vector.tensor_add(m21[:], m21[:], mv1[:, 1:2])
            nc.vector.tensor_scalar_mul(sums1[:, 1:2], m21[:], float(QSH))

            gst1 = gather8("st1", sums1, 2)
            # only a1 = g1*rsqrt(var+eps) matters: the spatial softmax is
            # invariant to the BN_1 shift (and to bt1)
            a1, _b1 = bn_affine(gst1, float(B * HWPIX), V[:, 2:3], V[:, 3:4])

            # E1 = exp(a1 * z1), with per-chunk sums from the ACT pass
            E1 = big.tile([C, NQT, TW], bf16)
            esums = small.tile([C, NQT], f32)
            for t in range(NQT):
                nc.scalar.activation(E1[:, t, :], z1[:, t, :], AF.Exp,
                                     scale=a1[:],
                                     accum_out=esums[:, t:t + 1])
            esum = small.tile([C, 1], f32)
            nc.vector.tensor_reduce(esum[:], esums[:], axis=AX.X, op=OP.add)

            ges = gather8("es", esum, 1)
            # this core's spatial-softmax denominator: the two partials of
            # its own batch pair, selected by the per-core one-hot mask
            gsel = small.tile([C, 8], f32)
            nc.vector.tensor_mul(gsel[:], ges[:, :, 0], Msk[:])
            den = small.tile([C, 1], f32)
            nc.vector.tensor_reduce(den[:], gsel[:], axis=AX.X, op=OP.add)
            rb = small.tile([C, 1], f32)
            nc.vector.reciprocal(rb[:], den[:])

            # y = E1 * rb (in place), then CBL_O conv + batch stats
            zO = big.tile([C, NQT, TW], bf16)
            stO = small.tile([C, 2 * NQT, 6], f32)
            for t in range(NQT):
                with nc.allow_low_precision(reason="softmax weights bf16"):
                    nc.vector.tensor_scalar_mul(E1[:, t, :], E1[:, t, :],
                                                rb[:])
                pzo = ps.tile([C, TW], f32, tag="ps")
                nc.tensor.matmul(pzo[:], Wo_s[:], E1[:, t, :],
                                 start=True, stop=True)
                nc.vector.bn_stats(stO[:, 2 * t, :], pzo[:, :512])
                nc.vector.bn_stats(stO[:, 2 * t + 1, :], pzo[:, 512:])
                nc.gpsimd.tensor_copy(zO[:, t, :], pzo[:])

            mvO = small.tile([C, 2], f32)
            nc.vector.bn_aggr(mvO[:], stO[:])
            sumsO = small.tile([C, 2], f32)
            nc.vector.tensor_scalar_mul(sumsO[:, 0:1], mvO[:, 0:1], float(QSH))
            m2O = small.tile([C, 1], f32)
            nc.vector.tensor_mul(m2O[:], mvO[:, 0:1], mvO[:, 0:1])
            nc.vector.tensor_add(m2O[:], m2O[:], mvO[:, 1:2])
            nc.vector.tensor_scalar_mul(sumsO[:, 1:2], m2O[:], float(QSH))

            gstO = gather8("stO", sumsO, 2)
            aO, bO = bn_affine(gstO, float(B * HWPIX), V[:, 4:5], V[:, 5:6])

            OUT = big.tile([C, NQT, TW], f32)
            for t in range(NQT):
                tmp = tmp2p.tile([C, TW], f32, tag="tmp2")
                nc.scalar.activation(tmp[:], zO[:, t, :], AF.Identity,
                                     scale=aO[:], bias=bO[:])
                nc.vector.scalar_tensor_tensor(OUT[:, t, :], tmp[:], ALPHA,
                                               tmp[:], op0=OP.mult,
                                               op1=OP.max)
                eng = nc.sync if t % 2 == 0 else nc.scalar
                eng.dma_start(d_outT[:, t * TW:(t + 1) * TW], OUT[:, t, :])

    nc.compile()
    return nc


def _get_runner():
    if "runner" in _CACHE:
        return _CACHE["runner"]
    import jax
    import numpy as np
    from jax.sharding import Mesh, PartitionSpec
    from jax.experimental.shard_map import shard_map
    from concourse import mybir
    from concourse.bass2jax import (_bass_exec_p, install_neuronx_cc_hook,
                                    partition_id_tensor)

    nc = _build_program()
    install_neuronx_cc_hook()

    in_names, out_names, out_avals, zero_outs = [], [], [], []
    partition_name = nc.partition_id_tensor.name if nc.partition_id_tensor else None
    for alloc in nc.m.functions[0].allocations:
        if not isinstance(alloc, mybir.MemoryLocationSet):
            continue
        name = alloc.memorylocations[0].name
        if alloc.kind == "ExternalInput":
            if name != partition_name:
                in_names.append(name)
        elif alloc.kind == "ExternalOutput":
            shape = tuple(alloc.tensor_shape)
            dtype = mybir.dt.np(alloc.dtype)
            out_names.append(name)
            out_avals.append(jax.core.ShapedArray(shape, dtype))
            zero_outs.append(np.zeros(shape, dtype))
    n_params = len(in_names)
    n_outs = len(out_avals)
    all_in_names = list(in_names) + list(out_names)
    if partition_name is not None:
        all_in_names.append(partition_name)

    def _body(*args):
        operands = list(args)
        if partition_name is not None:
            operands.append(partition_id_tensor())
        outs = _bass_exec_p.bind(
            *operands,
            out_avals=tuple(out_avals),
            in_names=tuple(all_in_names),
            out_names=tuple(out_names),
            lowering_input_output_aliases=(),
            sim_require_finite=True,
            sim_require_nnan=True,
            nc=nc,
        )
        return tuple(outs)

    donate = tuple(range(n_params, n_params + n_outs))
    try:
        devices = jax.devices("axon")[:N_CORES]
    except RuntimeError:
        devices = jax.devices()[:N_CORES]
    mesh = Mesh(np.asarray(devices), ("core",))
    in_specs = (PartitionSpec("core"),) * (n_params + n_outs)
    out_specs = (PartitionSpec("core"),) * n_outs
    sharded = jax.jit(
        shard_map(_body, mesh=mesh, in_specs=in_specs, out_specs=out_specs,
                  check_rep=False),
        donate_argnums=donate, keep_unused=True)

    def run(in_maps):
        per_core = [[np.asarray(m[name]) for name in in_names] for m in in_maps]
        concat_in = [np.concatenate([per_core[c][i] for c in range(N_CORES)],
                                    axis=0) for i in range(n_params)]
        concat_zeros = [np.zeros((N_CORES * z.shape[0], *z.shape[1:]), z.dtype)
                        for z in zero_outs]
        out_arrs = jax.block_until_ready(sharded(*concat_in, *concat_zeros))
        return [
            {name: np.asarray(out_arrs[i]).reshape(N_CORES, *out_avals[i].shape)[c]
             for i, name in enumerate(out_names)}
            for c in range(N_CORES)
        ]

    _CACHE["runner"] = run
    return run


def _make_blockmap():
    bm = np.zeros((C, C), np.float32)
    idx = np.arange(C)
    bm[(idx[:, None] // 64) == (idx[None, :] // 64)] = 1.0
    return bm


def kernel(x, Wq, bq, gq, btq, Wk, bk, g1, bt1, Wo, bo, go, bto):
    """Full inputs -> full output. Conv biases cancel inside training-mode
    BN (the mean subtraction removes any per-channel constant), so bq/bk/bo
    never enter the device program."""
    import ml_dtypes
    bf16 = ml_dtypes.bfloat16

    x = np.asarray(x, np.float32)
    run = _get_runner()

    wq9 = np.ascontiguousarray(
        np.asarray(Wq, np.float32).reshape(9, C, C)).astype(bf16)
    wk = np.ascontiguousarray(
        np.asarray(Wk, np.float32).reshape(C, C)).astype(bf16)
    wo = np.ascontiguousarray(
        np.asarray(Wo, np.float32).reshape(C, C)).astype(bf16)
    vecs = np.ascontiguousarray(np.stack([
        np.asarray(v, np.float32) for v in (gq, btq, g1, bt1, go, bto)]))
    bm = _make_blockmap().astype(bf16)

    # block-major key permutation: index (n,m,p,q) -> pixel (8n+p, 8m+q)
    perm = np.arange(HWPIX).reshape(8, 8, 8, 8).transpose(0, 2, 1, 3).reshape(-1)

    in_maps = []
    for core in range(N_CORES):
        b, h = core // 2, core % 2
        xb = np.ascontiguousarray(x[b].reshape(HWPIX, C))
        xbT = xb.T  # [C, HWPIX]
        xqT = np.ascontiguousarray(xbT[:, h * QSH:(h + 1) * QSH]).astype(bf16)
        xpadT = np.zeros((C, H + 2, W + 2), np.float32)
        xpadT[:, 1:H + 1, 1:W + 1] = xbT.reshape(C, H, W)
        msk = np.zeros((8, C), np.float32)
        msk[2 * b] = 1.0
        msk[2 * b + 1] = 1.0
        in_maps.append({
            "xnat": np.ascontiguousarray(xb[perm]).astype(bf16),
            "xqT": xqT,
            "xpadT": np.ascontiguousarray(
                xpadT.reshape(C, PADN)).astype(bf16),
            "wq9": wq9, "wk": wk, "wo": wo, "vecs": vecs, "msk": msk,
            "bm": bm,
        })

    res = run(in_maps)
    full = np.empty((B, HWPIX, C), np.float32)
    for core in range(N_CORES):
        b, h = core // 2, core % 2
        full[b, h * QSH:(h + 1) * QSH, :] = res[core]["outT"].T
    return full.reshape(B, H, W, C)


# revision 6
# speedup vs baseline: 2.3474x; 1.1199x over previous
"""Trainium2 Bass kernel for nn_GroupAttentionLayer (sparse block attention).

Strategy (8 NeuronCores, SPMD): core i handles batch b=i//2, query-pixel
half h=i%2 (2048 query pixels each). All heavy tensors are bf16 (PSUM
accumulation stays fp32), attention tiles are 1024 queries wide, and the
epilogue is fully SHARDED (the 1x1 conv + BN/softmax reductions are
pointwise, so no core ever needs the full tensor):

    scores^T[k,q] = Qc[:,k].T @ Xq[:,q]      (PE, contract channels, bf16)
    E = exp(scores/8)                        (ACT, bf16 out)
    D_bcast = blockmap.T @ E                 (PE, per-64-block sums)
    A = E / D_bcast                          (DVE tensor_tensor divide,
                                              every POOL_EVERY-th on Pool)
    agg^T[c,q] += x_blk[k,:].T @ A           (PE, PSUM accum, Conv_K folded)

The PE instruction stream is software-pipelined (scores issued 2 iters
ahead, blockmap 1 ahead) so the in-order engine queues never stall on
cross-engine dependencies.

Four tiny AllGathers (cheaper than AllReduce under the collective cost
model) carry the cross-core reductions: BN_Q batch stats, BN_1 batch
stats, per-batch spatial-softmax exp sums, BN_O batch stats. BN shifts
that feed the spatial softmax cancel algebraically (softmax is
shift-invariant) and conv biases cancel inside training-mode BN, so
neither is computed. A per-core one-hot mask input selects this core's
batch pair out of the gathered exp sums.

Host side: shards/transposes/bf16-converts inputs with numpy, assembles
the output from the 8 per-core channel-major shards.
"""

import numpy as np

B, H, W, C = 4, 64, 64, 128
RF = 8
EPS = 1e-3
ALPHA = 0.1
N_CORES = 8
HWPIX = H * W             # 4096 pixels per batch
QSH = HWPIX * B // N_CORES  # 2048 query pixels per core
PW = W + 2                # 66, padded row width
PADN = PW * (H + 2)       # 4356 padded columns
NKT = HWPIX // 128        # 32 key tiles per batch
TW = 1024                 # attention tile width (queries)
NQT = QSH // TW           # 2 query tiles per core
NCC = HWPIX // TW         # 4 conv chunks (2 block-rows each)
NIT = NQT * NKT           # 64 inner iterations

# every POOL_EVERY-th divide runs on Pool (gpsimd), the rest on DVE.
# 0 disables the Pool share (DVE has slack; fewer semaphores)
POOL_EVERY = 0

_CACHE = {}


def _build_program():
    import concourse.bacc as bacc
    import concourse.tile as tile
    from concourse import mybir

    f32 = mybir.dt.float32
    bf16 = mybir.dt.bfloat16
    AF = mybir.ActivationFunctionType
    OP = mybir.AluOpType
    AX = mybir.AxisListType

    nc = bacc.Bacc("TRN2", target_bir_lowering=False, debug=False,
                   enable_asserts=True, num_devices=N_CORES)

    # per-core inputs (bf16 heavy data, fp32 vectors)
    d_xnat = nc.dram_tensor("xnat", [HWPIX, C], bf16, kind="ExternalInput").ap()
    d_xqT = nc.dram_tensor("xqT", [C, QSH], bf16, kind="ExternalInput").ap()
    d_xpadT = nc.dram_tensor("xpadT", [C, PADN], bf16, kind="ExternalInput").ap()
    d_wq9 = nc.dram_tensor("wq9", [9, C, C], bf16, kind="ExternalInput").ap()
    d_wk = nc.dram_tensor("wk", [C, C], bf16, kind="ExternalInput").ap()
    d_wo = nc.dram_tensor("wo", [C, C], bf16, kind="ExternalInput").ap()
    d_vecs = nc.dram_tensor("vecs", [6, C], f32, kind="ExternalInput").ap()
    d_msk = nc.dram_tensor("msk", [8, C], f32, kind="ExternalInput").ap()
    d_bm = nc.dram_tensor("bm", [C, C], bf16, kind="ExternalInput").ap()
    # output: this core's channel-major shard
    d_outT = nc.dram_tensor("outT", [C, QSH], f32, kind="ExternalOutput").ap()

    with tile.TileContext(nc) as tc:
        with tc.tile_pool(name="const", bufs=1) as const, \
             tc.tile_pool(name="big", bufs=1) as big, \
             tc.tile_pool(name="work", bufs=3) as work, \
             tc.tile_pool(name="tmp2", bufs=2) as tmp2p, \
             tc.tile_pool(name="small", bufs=2) as small, \
             tc.tile_pool(name="ps", bufs=3, space="PSUM") as ps, \
             tc.tile_pool(name="psA", bufs=1, space="PSUM") as psA, \
             tc.tile_pool(name="dram", bufs=1, space="DRAM") as dram:

            # ---------------- loads ----------------
            # conv inputs (Wq + Xpad row-chunks) first on their queues so the
            # conv can start ~3us in; everything else behind them
            Wq_s = const.tile([C, 9, C], bf16)
            nc.scalar.dma_start(
                Wq_s[:], d_wq9.rearrange("t ci co -> ci t co"))
            Xpad = big.tile([C, PADN], bf16)
            Xpr = Xpad[:].rearrange("p (r c) -> p r c", r=H + 2)
            d_xpr = d_xpadT.rearrange("p (r c) -> p r c", r=H + 2)
            for rr in range(4):
                r0, r1 = [(0, 18), (18, 34), (34, 50), (50, 66)][rr]
                nc.sync.dma_start(Xpr[:, r0:r1, :], d_xpr[:, r0:r1, :])
            Xq = big.tile([C, QSH], bf16)
            nc.sync.dma_start(Xq[:], d_xqT[:])
            Xnat = big.tile([128, NKT, C], bf16)
            nc.scalar.dma_start(
                Xnat[:], d_xnat.rearrange("(t p) c -> p t c", p=128))
            Wk_s = const.tile([C, C], bf16)
            nc.sync.dma_start(Wk_s[:], d_wk[:])
            Wo_s = const.tile([C, C], bf16)
            nc.sync.dma_start(Wo_s[:], d_wo[:])
            V = const.tile([C, 6], f32)
            nc.scalar.dma_start(V[:], d_vecs.rearrange("v c -> c v"))
            Msk = const.tile([C, 8], f32)
            nc.scalar.dma_start(Msk[:], d_msk.rearrange("v c -> c v"))
            Bb = const.tile([C, C], bf16)
            nc.sync.dma_start(Bb[:], d_bm[:])
            eps_t = const.tile([C, 1], f32)
            nc.vector.memset(eps_t[:], EPS)
            Wz = const.tile([C, TW], bf16)
            nc.vector.memset(Wz[:], 0.0)

            def warm(n):
                # dependency-free matmuls that keep the PE pstate at full
                # clock through windows where real work is blocked
                for _ in range(n):
                    wp = ps.tile([C, TW], f32, tag="ps")
                    nc.tensor.matmul(wp[:], Wz[:, :C], Wz[:],
                                     start=True, stop=True)

            warm(10)

            Xpv = Xpad[:].rearrange("p (r c) -> p r c", r=H + 2)

            # ---------------- CBL_Q: conv3x3 + batch stats ----------------
            # Conv output in BLOCK-MAJOR key order: chunk t covers block rows
            # n=2t,2t+1; column n*512 + m*64 + p*8 + q is pixel (8n+p, 8m+q).
            Zq = big.tile([C, NCC, TW], bf16)
            qstats = small.tile([C, 2 * NCC, 6], f32)
            for t in range(NCC):
                pq = ps.tile([C, TW], f32, tag="ps")
                for tap in range(9):
                    dh, dw = tap // 3 - 1, tap % 3 - 1
                    rhs = Xpv[:, t * 16 + 1 + dh: t * 16 + 17 + dh,
                              1 + dw: 65 + dw].rearrange(
                                  "c (n p) (m q) -> c n m p q", n=2, m=8)
                    nc.tensor.matmul(pq[:], Wq_s[:, tap, :], rhs,
                                     start=(tap == 0), stop=(tap == 8))
                nc.vector.bn_stats(qstats[:, 2 * t, :], pq[:, :512])
                nc.vector.bn_stats(qstats[:, 2 * t + 1, :], pq[:, 512:])
                nc.gpsimd.tensor_copy(Zq[:, t, :], pq[:])

            qmv = small.tile([C, 2], f32)
            nc.vector.bn_aggr(qmv[:], qstats[:])
            # partial sums for the cross-core stats:
            #   sums[:,0] = mean * 4096 ; sums[:,1] = (var + mean^2) * 4096
            sums = small.tile([C, 2], f32)
            nc.vector.tensor_scalar_mul(sums[:, 0:1], qmv[:, 0:1], float(HWPIX))
            m2 = small.tile([C, 1], f32)
            nc.vector.tensor_mul(m2[:], qmv[:, 0:1], qmv[:, 0:1])
            nc.vector.tensor_add(m2[:], m2[:], qmv[:, 1:2])
            nc.vector.tensor_scalar_mul(sums[:, 1:2], m2[:], float(HWPIX))

            grp = [list(range(N_CORES))]

            def gather8(name, src, width, warm_n=0):
                """AllGather a [C,width] fp32 tile -> [C,8,width] on-chip."""
                t_in = dram.tile([C, width], f32, tag=f"{name}_in")
                t_out = dram.tile([N_CORES * C, width], f32,
                                  addr_space="Shared", tag=f"{name}_out")
                nc.sync.dma_start(t_in[:], src[:])
                nc.gpsimd.collective_compute(
                    "AllGather", OP.bypass, replica_groups=grp,
                    ins=[t_in.opt()], outs=[t_out.opt()])
                if warm_n:
                    warm(warm_n)
                g = small.tile([C, 8, width], f32, tag=f"{name}_g")
                nc.sync.dma_start(
                    g[:], t_out[:].rearrange("(r p) s -> p r s", r=N_CORES))
                return g

            def bn_affine(gst8, tot, gamma, beta=None):
                """gst8: [C,8,2] gathered (sum, sumsq) partials. Returns
                (a, b): a = gamma*rsqrt(var+eps), b = beta - a*mean (b=None
                when beta is None -- the shift cancels downstream)."""
                gsum = small.tile([C, 2], f32, tag="gsum")
                nc.vector.tensor_reduce(
                    gsum[:], gst8[:].rearrange("c r s -> c s r"),
                    axis=AX.X, op=OP.add)
                sc = small.tile([C, 2], f32, tag="scmom")
                nc.vector.tensor_scalar_mul(sc[:], gsum[:], 1.0 / tot)
                negvar = small.tile([C, 1], f32, tag="negvar")
                nc.vector.scalar_tensor_tensor(negvar[:], sc[:, 0:1],
                                               sc[:, 0:1], sc[:, 1:2],
                                               op0=OP.mult, op1=OP.subtract)
                std = small.tile([C, 1], f32, tag="std")
                nc.scalar.activation(std[:], negvar[:], AF.Sqrt,
                                     scale=-1.0, bias=eps_t[:])
                a = small.tile([C, 1], f32, tag="acoef")
                nc.vector.tensor_tensor(a[:], gamma, std[:], op=OP.divide)
                if beta is None:
                    return a, None
                b = small.tile([C, 1], f32, tag="bcoef")
                nc.vector.tensor_scalar(b[:], sc[:, 0:1], a[:], -1.0,
                                        op0=OP.mult, op1=OP.mult)
                nc.vector.tensor_add(b[:], b[:], beta)
                return a, b

            gstq = gather8("stq", sums, 2, warm_n=30)
            aq, bq = bn_affine(gstq, float(HWPIX * N_CORES),
                               V[:, 0:1], V[:, 1:2])

            # q = leaky(aq*z + bq); Zq is already block-major
            Qc = big.tile([C, HWPIX], bf16)
            Qv = Qc[:].rearrange("p (t f) -> p t f", f=TW)
            for t in range(NCC):
                tmp = tmp2p.tile([C, TW], f32, tag="tmp2")
                nc.scalar.activation(tmp[:], Zq[:, t, :], AF.Identity,
                                     scale=aq[:], bias=bq[:])
                nc.vector.scalar_tensor_tensor(Qv[:, t, :], tmp[:], ALPHA,
                                               tmp[:], op0=OP.mult,
                                               op1=OP.max)

            # ---------------- attention main loop (software-pipelined) ----
            z1 = big.tile([C, NQT, TW], bf16)
            qs1 = small.tile([C, 2 * NQT, 6], f32)
            paggs = [None] * NQT
            Es = [None] * (NIT + 2)
            with nc.allow_low_precision(reason="attn weights in bf16"):
                for i in range(NIT + 2):
                    qt, kt = i // NKT, i % NKT
                    # stage 0: scores (+ per-qt Conv_K group start)
                    if i < NIT:
                        if kt == 0 and qt == 0:
                            pagg0 = psA.tile([C, TW], f32, tag="agg")
                            paggs[0] = pagg0
                            nc.tensor.matmul(pagg0[:], Wk_s[:],
                                             Xq[:, :TW],
                                             start=True, stop=False)
                        if kt == 2 and qt + 1 < NQT:
                            # issued just after agg[qt,31] so the single
                            # psA slot never deadlocks the in-order PE queue
                            pagg_n = psA.tile([C, TW], f32, tag="agg")
                            paggs[qt + 1] = pagg_n
                            nc.tensor.matmul(
                                pagg_n[:], Wk_s[:],
                                Xq[:, (qt + 1) * TW:(qt + 2) * TW],
                                start=True, stop=False)
                        psS = ps.tile([C, TW], f32, tag="ps")
                        nc.tensor.matmul(psS[:],
                                         Qc[:, kt * 128:(kt + 1) * 128],
                                         Xq[:, qt * TW:(qt + 1) * TW],
                                         start=True, stop=True)
                        Ei = work.tile([C, TW], bf16, tag="E")
                        Es[i] = Ei
                        nc.scalar.activation(Ei[:], psS[:], AF.Exp,
                                             scale=1.0 / RF)
                    # stage 1: block-sum matmul + divide (A = E/D in place)
                    if 1 <= i < NIT + 1:
                        j = i - 1
                        psD = ps.tile([C, TW], f32, tag="ps")
                        nc.tensor.matmul(psD[:], Bb[:], Es[j][:],
                                         start=True, stop=True)
                        eng = nc.gpsimd if (POOL_EVERY and
                                            j % POOL_EVERY == POOL_EVERY - 1) \
                            else nc.vector
                        eng.tensor_tensor(Es[j][:], Es[j][:], psD[:],
                                          op=OP.divide)
                    # stage 2: weighted-sum accumulate
                    if i >= 2:
                        j = i - 2
                        jqt, jkt = j // NKT, j % NKT
                        nc.tensor.matmul(paggs[jqt][:], Xnat[:, jkt, :],
                                         Es[j][:], start=False,
                                         stop=(jkt == NKT - 1))
                        Es[j] = None
                        if jkt == NKT - 1:
                            nc.vector.bn_stats(qs1[:, 2 * jqt, :],
                                               paggs[jqt][:, :512])
                            nc.vector.bn_stats(qs1[:, 2 * jqt + 1, :],
                                               paggs[jqt][:, 512:])
                            nc.gpsimd.tensor_copy(z1[:, jqt, :],
                                                  paggs[jqt][:])
                            paggs[jqt] = None

            # ---------------- epilogue (sharded) ----------------
            # partial BN_1 sums for this core's 2048 pixels
            mv1 = small.tile([C, 2], f32)
            nc.vector.bn_aggr(mv1[:], qs1[:])
            sums1 = small.tile([C, 2], f32)
            nc.vector.tensor_scalar_mul(sums1[:, 0:1], mv1[:, 0:1], float(QSH))
            m21 = small.tile([C, 1], f32)
            nc.vector.tensor_mul(m21[:], mv1[:, 0:1], mv1[:, 0:1])
            nc.vector.tensor_add(m21[:], m21[:], mv1[:, 1:2])
            nc.vector.tensor_scalar_mul(sums1[:, 1:2], m21[:], float(QSH))

            gst1 = gather8("st1", sums1, 2, warm_n=12)
            # only a1 = g1*rsqrt(var+eps) matters: the spatial softmax is
            # invariant to the BN_1 shift (and to bt1)
            a1, _b1 = bn_affine(gst1, float(B * HWPIX), V[:, 2:3])

            # E1 = exp(a1 * z1), with per-chunk sums from the ACT pass
            E1 = big.tile([C, NQT, TW], bf16)
            esums = small.tile([C, NQT], f32)
            for t in range(NQT):
                nc.scalar.activation(E1[:, t, :], z1[:, t, :], AF.Exp,
                                     scale=a1[:],
                                     accum_out=esums[:, t:t + 1])
            esum = small.tile([C, 1], f32)
            nc.vector.tensor_reduce(esum[:], esums[:], axis=AX.X, op=OP.add)

            ges = gather8("es", esum, 1, warm_n=12)
            # this core's spatial-softmax denominator: the two partials of
            # its own batch pair, selected by the per-core one-hot mask
            gsel = small.tile([C, 8], f32)
            nc.vector.tensor_mul(gsel[:], ges[:, :, 0], Msk[:])
            den = small.tile([C, 1], f32)
            nc.vector.tensor_reduce(den[:], gsel[:], axis=AX.X, op=OP.add)
            rb = small.tile([C, 1], f32)
            nc.vector.reciprocal(rb[:], den[:])

            # y = E1 * rb (in place), then CBL_O conv + batch stats
            zO = big.tile([C, NQT, TW], bf16)
            stO = small.tile([C, 2 * NQT, 6], f32)
            for t in range(NQT):
                with nc.allow_low_precision(reason="softmax weights bf16"):
                    nc.vector.tensor_scalar_mul(E1[:, t, :], E1[:, t, :],
                                                rb[:])
                pzo = ps.tile([C, TW], f32, tag="ps")
                nc.tensor.matmul(pzo[:], Wo_s[:], E1[:, t, :],
                                 start=True, stop=True)
                nc.vector.bn_stats(stO[:, 2 * t, :], pzo[:, :512])
                nc.vector.bn_stats(stO[:, 2 * t + 1, :], pzo[:, 512:])
                nc.gpsimd.tensor_copy(zO[:, t, :], pzo[:])

            mvO = small.tile([C, 2], f32)
            nc.vector.bn_aggr(mvO[:], stO[:])
            sumsO = small.tile([C, 2], f32)
            nc.vector.tensor_scalar_mul(sumsO[:, 0:1], mvO[:, 0:1], float(QSH))
            m2O = small.tile([C, 1], f32)
            nc.vector.tensor_mul(m2O[:], mvO[:, 0:1], mvO[:, 0:1])
            nc.vector.tensor_add(m2O[:], m2O[:], mvO[:, 1:2])
            nc.vector.tensor_scalar_mul(sumsO[:, 1:2], m2O[:], float(QSH))

            gstO = gather8("stO", sumsO, 2)
            aO, bO = bn_affine(gstO, float(B * HWPIX), V[:, 4:5], V[:, 5:6])

            OUT = big.tile([C, NQT, TW], f32)
            for t in range(NQT):
                tmp = tmp2p.tile([C, TW], f32, tag="tmp2")
                nc.scalar.activation(tmp[:], zO[:, t, :], AF.Identity,
                                     scale=aO[:], bias=bO[:])
                nc.vector.scalar_tensor_tensor(OUT[:, t, :], tmp[:], ALPHA,
                                               tmp[:], op0=OP.mult,
                                               op1=OP.max)
                eng = nc.sync if t % 2 == 0 else nc.scalar
                eng.dma_start(d_outT[:, t * TW:(t + 1) * TW], OUT[:, t, :])

    nc.compile()
    return nc


def _get_runner():
    if "runner" in _CACHE:
        return _CACHE["runner"]
    import jax
    import numpy as np
    from jax.sharding import Mesh, PartitionSpec
    from jax.experimental.shard_map import shard_map
    from concourse import mybir
    from concourse.bass2jax import (_bass_exec_p, install_neuronx_cc_hook,
                                    partition_id_tensor)

    nc = _build_program()
    install_neuronx_cc_hook()

    in_names, out_names, out_avals, zero_outs = [], [], [], []
    partition_name = nc.partition_id_tensor.name if nc.partition_id_tensor else None
    for alloc in nc.m.functions[0].allocations:
        if not isinstance(alloc, mybir.MemoryLocationSet):
            continue
        name = alloc.memorylocations[0].name
        if alloc.kind == "ExternalInput":
            if name != partition_name:
                in_names.append(name)
        elif alloc.kind == "ExternalOutput":
            shape = tuple(alloc.tensor_shape)
            dtype = mybir.dt.np(alloc.dtype)
            out_names.append(name)
            out_avals.append(jax.core.ShapedArray(shape, dtype))
            zero_outs.append(np.zeros(shape, dtype))
    n_params = len(in_names)
    n_outs = len(out_avals)
    all_in_names = list(in_names) + list(out_names)
    if partition_name is not None:
        all_in_names.append(partition_name)

    def _body(*args):
        operands = list(args)
        if partition_name is not None:
            operands.append(partition_id_tensor())
        outs = _bass_exec_p.bind(
            *operands,
            out_avals=tuple(out_avals),
            in_names=tuple(all_in_names),
            out_names=tuple(out_names),
            lowering_input_output_aliases=(),
            sim_require_finite=True,
            sim_require_nnan=True,
            nc=nc,
        )
        return tuple(outs)

    donate = tuple(range(n_params, n_params + n_outs))
    try:
        devices = jax.devices("axon")[:N_CORES]
    except RuntimeError:
        devices = jax.devices()[:N_CORES]
    mesh = Mesh(np.asarray(devices), ("core",))
    in_specs = (PartitionSpec("core"),) * (n_params + n_outs)
    out_specs = (PartitionSpec("core"),) * n_outs
    sharded = jax.jit(
        shard_map(_body, mesh=mesh, in_specs=in_specs, out_specs=out_specs,
                  check_rep=False),
        donate_argnums=donate, keep_unused=True)

    def run(in_maps):
        per_core = [[np.asarray(m[name]) for name in in_names] for m in in_maps]
        concat_in = [np.concatenate([per_core[c][i] for c in range(N_CORES)],
                                    axis=0) for i in range(n_params)]
        concat_zeros = [np.zeros((N_CORES * z.shape[0], *z.shape[1:]), z.dtype)
                        for z in zero_outs]
        out_arrs = jax.block_until_ready(sharded(*concat_in, *concat_zeros))
        return [
            {name: np.asarray(out_arrs[i]).reshape(N_CORES, *out_avals[i].shape)[c]
             for i, name in enumerate(out_names)}
            for c in range(N_CORES)
        ]

    _CACHE["runner"] = run
    return run


def _make_blockmap():
    bm = np.zeros((C, C), np.float32)
    idx = np.arange(C)
    bm[(idx[:, None] // 64) == (idx[None, :] // 64)] = 1.0
    return bm


def kernel(x, Wq, bq, gq, btq, Wk, bk, g1, bt1, Wo, bo, go, bto):
    """Full inputs -> full output. Conv biases cancel inside training-mode
    BN (the mean subtraction removes any per-channel constant), so bq/bk/bo
    never enter the device program."""
    import ml_dtypes
    bf16 = ml_dtypes.bfloat16

    x = np.asarray(x, np.float32)
    run = _get_runner()

    wq9 = np.ascontiguousarray(
        np.asarray(Wq, np.float32).reshape(9, C, C)).astype(bf16)
    wk = np.ascontiguousarray(
        np.asarray(Wk, np.float32).reshape(C, C)).astype(bf16)
    wo = np.ascontiguousarray(
        np.asarray(Wo, np.float32).reshape(C, C)).astype(bf16)
    vecs = np.ascontiguousarray(np.stack([
        np.asarray(v, np.float32) for v in (gq, btq, g1, bt1, go, bto)]))
    bm = _make_blockmap().astype(bf16)

    # block-major key permutation: index (n,m,p,q) -> pixel (8n+p, 8m+q)
    perm = np.arange(HWPIX).reshape(8, 8, 8, 8).transpose(0, 2, 1, 3).reshape(-1)

    in_maps = []
    for core in range(N_CORES):
        b, h = core // 2, core % 2
        xb = np.ascontiguousarray(x[b].reshape(HWPIX, C))
        xbT = xb.T  # [C, HWPIX]
        xqT = np.ascontiguousarray(xbT[:, h * QSH:(h + 1) * QSH]).astype(bf16)
        xpadT = np.zeros((C, H + 2, W + 2), np.float32)
        xpadT[:, 1:H + 1, 1:W + 1] = xbT.reshape(C, H, W)
        msk = np.zeros((8, C), np.float32)
        msk[2 * b] = 1.0
        msk[2 * b + 1] = 1.0
        in_maps.append({
            "xnat": np.ascontiguousarray(xb[perm]).astype(bf16),
            "xqT": xqT,
            "xpadT": np.ascontiguousarray(
                xpadT.reshape(C, PADN)).astype(bf16),
            "wq9": wq9, "wk": wk, "wo": wo, "vecs": vecs, "msk": msk,
            "bm": bm,
        })

    res = run(in_maps)
    full = np.empty((B, HWPIX, C), np.float32)
    for core in range(N_CORES):
        b, h = core // 2, core % 2
        full[b, h * QSH:(h + 1) * QSH, :] = res[core]["outT"].T
    return full.reshape(B, H, W, C)


# revision 7
# speedup vs baseline: 2.3499x; 1.0011x over previous
"""Trainium2 Bass kernel for nn_GroupAttentionLayer (sparse block attention).

Strategy (8 NeuronCores, SPMD): core i handles batch b=i//2, query-pixel
half h=i%2 (2048 query pixels each). All heavy tensors are bf16 (PSUM
accumulation stays fp32), attention tiles are 1024 queries wide, and the
epilogue is fully SHARDED (the 1x1 conv + BN/softmax reductions are
pointwise, so no core ever needs the full tensor):

    scores^T[k,q] = Qc[:,k].T @ Xq[:,q]      (PE, contract channels, bf16)
    E = exp(scores/8)                        (ACT, bf16 out)
    D_bcast = blockmap.T @ E                 (PE, per-64-block sums)
    A = E / D_bcast                          (DVE tensor_tensor divide,
                                              every POOL_EVERY-th on Pool)
    agg^T[c,q] += x_blk[k,:].T @ A           (PE, PSUM accum, Conv_K folded)

The PE instruction stream is software-pipelined (scores issued 2 iters
ahead, blockmap 1 ahead) so the in-order engine queues never stall on
cross-engine dependencies.

Four tiny AllGathers (cheaper than AllReduce under the collective cost
model) carry the cross-core reductions: BN_Q batch stats, BN_1 batch
stats, per-batch spatial-softmax exp sums, BN_O batch stats. BN shifts
that feed the spatial softmax cancel algebraically (softmax is
shift-invariant) and conv biases cancel inside training-mode BN, so
neither is computed. A per-core one-hot mask input selects this core's
batch pair out of the gathered exp sums.

Host side: shards/transposes/bf16-converts inputs with numpy, assembles
the output from the 8 per-core channel-major shards.
"""

import numpy as np

B, H, W, C = 4, 64, 64, 128
RF = 8
EPS = 1e-3
ALPHA = 0.1
N_CORES = 8
HWPIX = H * W             # 4096 pixels per batch
QSH = HWPIX * B // N_CORES  # 2048 query pixels per core
PW = W + 2                # 66, padded row width
PADN = PW * (H + 2)       # 4356 padded columns
NKT = HWPIX // 128        # 32 key tiles per batch
TW = 1024                 # attention tile width (queries)
NQT = QSH // TW           # 2 query tiles per core
NCC = HWPIX // TW         # 4 conv chunks (2 block-rows each)
NIT = NQT * NKT           # 64 inner iterations

# every POOL_EVERY-th divide runs on Pool (gpsimd), the rest on DVE.
# 0 disables the Pool share (DVE has slack; fewer semaphores)
POOL_EVERY = 0

_CACHE = {}


def _build_program():
    import concourse.bacc as bacc
    import concourse.tile as tile
    from concourse import mybir

    f32 = mybir.dt.float32
    bf16 = mybir.dt.bfloat16
    AF = mybir.ActivationFunctionType
    OP = mybir.AluOpType
    AX = mybir.AxisListType

    nc = bacc.Bacc("TRN2", target_bir_lowering=False, debug=False,
                   enable_asserts=True, num_devices=N_CORES)

    # per-core inputs (bf16 heavy data, fp32 vectors)
    d_xnat = nc.dram_tensor("xnat", [HWPIX, C], bf16, kind="ExternalInput").ap()
    d_xqT = nc.dram_tensor("xqT", [C, QSH], bf16, kind="ExternalInput").ap()
    d_xpadT = nc.dram_tensor("xpadT", [C, PADN], bf16, kind="ExternalInput").ap()
    d_wq9 = nc.dram_tensor("wq9", [9, C, C], bf16, kind="ExternalInput").ap()
    d_wk = nc.dram_tensor("wk", [C, C], bf16, kind="ExternalInput").ap()
    d_wo = nc.dram_tensor("wo", [C, C], bf16, kind="ExternalInput").ap()
    d_vecs = nc.dram_tensor("vecs", [6, C], f32, kind="ExternalInput").ap()
    d_msk = nc.dram_tensor("msk", [8, C], f32, kind="ExternalInput").ap()
    d_bm = nc.dram_tensor("bm", [C, C], bf16, kind="ExternalInput").ap()
    # output: this core's channel-major shard
    d_outT = nc.dram_tensor("outT", [C, QSH], f32, kind="ExternalOutput").ap()

    with tile.TileContext(nc) as tc:
        with tc.tile_pool(name="const", bufs=1) as const, \
             tc.tile_pool(name="big", bufs=1) as big, \
             tc.tile_pool(name="work", bufs=3) as work, \
             tc.tile_pool(name="tmp2", bufs=2) as tmp2p, \
             tc.tile_pool(name="small", bufs=2) as small, \
             tc.tile_pool(name="ps", bufs=3, space="PSUM") as ps, \
             tc.tile_pool(name="psA", bufs=1, space="PSUM") as psA, \
             tc.tile_pool(name="dram", bufs=1, space="DRAM") as dram:

            # ---------------- loads ----------------
            # conv inputs (Wq + Xpad row-chunks) first on their queues so the
            # conv can start ~3us in; everything else behind them
            Wq_s = const.tile([C, 9, C], bf16)
            nc.scalar.dma_start(
                Wq_s[:], d_wq9.rearrange("t ci co -> ci t co"))
            Xpad = big.tile([C, PADN], bf16)
            Xpr = Xpad[:].rearrange("p (r c) -> p r c", r=H + 2)
            d_xpr = d_xpadT.rearrange("p (r c) -> p r c", r=H + 2)
            for rr in range(4):
                r0, r1 = [(0, 18), (18, 34), (34, 50), (50, 66)][rr]
                nc.sync.dma_start(Xpr[:, r0:r1, :], d_xpr[:, r0:r1, :])
            Xq = big.tile([C, QSH], bf16)
            nc.sync.dma_start(Xq[:], d_xqT[:])
            Xnat = big.tile([128, NKT, C], bf16)
            nc.scalar.dma_start(
                Xnat[:], d_xnat.rearrange("(t p) c -> p t c", p=128))
            Wk_s = const.tile([C, C], bf16)
            nc.sync.dma_start(Wk_s[:], d_wk[:])
            Wo_s = const.tile([C, C], bf16)
            nc.sync.dma_start(Wo_s[:], d_wo[:])
            V = const.tile([C, 6], f32)
            nc.scalar.dma_start(V[:], d_vecs.rearrange("v c -> c v"))
            Msk = const.tile([C, 8], f32)
            nc.scalar.dma_start(Msk[:], d_msk.rearrange("v c -> c v"))
            Bb = const.tile([C, C], bf16)
            nc.sync.dma_start(Bb[:], d_bm[:])
            eps_t = const.tile([C, 1], f32)
            nc.vector.memset(eps_t[:], EPS)
            Wz = const.tile([C, TW], bf16)
            nc.vector.memset(Wz[:], 0.0)

            def warm(n):
                # dependency-free matmuls that keep the PE pstate at full
                # clock through windows where real work is blocked
                for _ in range(n):
                    wp = ps.tile([C, TW], f32, tag="ps")
                    nc.tensor.matmul(wp[:], Wz[:, :C], Wz[:],
                                     start=True, stop=True)

            warm(10)

            Xpv = Xpad[:].rearrange("p (r c) -> p r c", r=H + 2)

            # ---------------- CBL_Q: conv3x3 + batch stats ----------------
            # Conv output in BLOCK-MAJOR key order: chunk t covers block rows
            # n=2t,2t+1; column n*512 + m*64 + p*8 + q is pixel (8n+p, 8m+q).
            Zq = big.tile([C, NCC, TW], bf16)
            qstats = small.tile([C, 2 * NCC, 6], f32)
            for t in range(NCC):
                pq = ps.tile([C, TW], f32, tag="ps")
                for tap in range(9):
                    dh, dw = tap // 3 - 1, tap % 3 - 1
                    rhs = Xpv[:, t * 16 + 1 + dh: t * 16 + 17 + dh,
                              1 + dw: 65 + dw].rearrange(
                                  "c (n p) (m q) -> c n m p q", n=2, m=8)
                    nc.tensor.matmul(pq[:], Wq_s[:, tap, :], rhs,
                                     start=(tap == 0), stop=(tap == 8))
                nc.vector.bn_stats(qstats[:, 2 * t, :], pq[:, :512])
                nc.vector.bn_stats(qstats[:, 2 * t + 1, :], pq[:, 512:])
                nc.gpsimd.tensor_copy(Zq[:, t, :], pq[:])

            qmv = small.tile([C, 2], f32)
            nc.vector.bn_aggr(qmv[:], qstats[:])
            # partial sums for the cross-core stats:
            #   sums[:,0] = mean * 4096 ; sums[:,1] = (var + mean^2) * 4096
            sums = small.tile([C, 2], f32)
            nc.vector.tensor_scalar_mul(sums[:, 0:1], qmv[:, 0:1], float(HWPIX))
            m2 = small.tile([C, 1], f32)
            nc.vector.tensor_mul(m2[:], qmv[:, 0:1], qmv[:, 0:1])
            nc.vector.tensor_add(m2[:], m2[:], qmv[:, 1:2])
            nc.vector.tensor_scalar_mul(sums[:, 1:2], m2[:], float(HWPIX))

            grp = [list(range(N_CORES))]

            def gather8(name, src, width, warm_n=0):
                """AllGather a [C,width] fp32 tile -> [C,8,width] on-chip."""
                t_in = dram.tile([C, width], f32, tag=f"{name}_in")
                t_out = dram.tile([N_CORES * C, width], f32,
                                  addr_space="Shared", tag=f"{name}_out")
                nc.sync.dma_start(t_in[:], src[:])
                nc.gpsimd.collective_compute(
                    "AllGather", OP.bypass, replica_groups=grp,
                    ins=[t_in.opt()], outs=[t_out.opt()])
                if warm_n:
                    warm(warm_n)
                g = small.tile([C, 8, width], f32, tag=f"{name}_g")
                nc.sync.dma_start(
                    g[:], t_out[:].rearrange("(r p) s -> p r s", r=N_CORES))
                return g

            def bn_affine(gst8, tot, gamma, beta=None):
                """gst8: [C,8,2] gathered (sum, sumsq) partials. Returns
                (a, b): a = gamma*rsqrt(var+eps), b = beta - a*mean (b=None
                when beta is None -- the shift cancels downstream)."""
                gsum = small.tile([C, 2], f32, tag="gsum")
                nc.vector.tensor_reduce(
                    gsum[:], gst8[:].rearrange("c r s -> c s r"),
                    axis=AX.X, op=OP.add)
                sc = small.tile([C, 2], f32, tag="scmom")
                nc.vector.tensor_scalar_mul(sc[:], gsum[:], 1.0 / tot)
                negvar = small.tile([C, 1], f32, tag="negvar")
                nc.vector.scalar_tensor_tensor(negvar[:], sc[:, 0:1],
                                               sc[:, 0:1], sc[:, 1:2],
                                               op0=OP.mult, op1=OP.subtract)
                # rsqrt(var+eps) = exp(-0.5*ln(var+eps)); ln/exp share one
                # ACT table set, so no LoadActFuncSet in this chain
                lnv = small.tile([C, 1], f32, tag="lnv")
                nc.scalar.activation(lnv[:], negvar[:], AF.Ln,
                                     scale=-1.0, bias=eps_t[:])
                rstd = small.tile([C, 1], f32, tag="rstd")
                nc.scalar.activation(rstd[:], lnv[:], AF.Exp, scale=-0.5)
                a = small.tile([C, 1], f32, tag="acoef")
                nc.vector.tensor_mul(a[:], rstd[:], gamma)
                if beta is None:
                    return a, None
                b = small.tile([C, 1], f32, tag="bcoef")
                nc.vector.tensor_scalar(b[:], sc[:, 0:1], a[:], -1.0,
                                        op0=OP.mult, op1=OP.mult)
                nc.vector.tensor_add(b[:], b[:], beta)
                return a, b

            gstq = gather8("stq", sums, 2, warm_n=46)
            aq, bq = bn_affine(gstq, float(HWPIX * N_CORES),
                               V[:, 0:1], V[:, 1:2])

            # q = leaky(aq*z + bq); Zq is already block-major
            Qc = big.tile([C, HWPIX], bf16)
            Qv = Qc[:].rearrange("p (t f) -> p t f", f=TW)
            for t in range(NCC):
                tmp = tmp2p.tile([C, TW], f32, tag="tmp2")
                nc.scalar.activation(tmp[:], Zq[:, t, :], AF.Identity,
                                     scale=aq[:], bias=bq[:])
                nc.vector.scalar_tensor_tensor(Qv[:, t, :], tmp[:], ALPHA,
                                               tmp[:], op0=OP.mult,
                                               op1=OP.max)

            # ---------------- attention main loop (software-pipelined) ----
            z1 = big.tile([C, NQT, TW], bf16)
            qs1 = small.tile([C, 2 * NQT, 6], f32)
            paggs = [None] * NQT
            Es = [None] * (NIT + 2)
            with nc.allow_low_precision(reason="attn weights in bf16"):
                for i in range(NIT + 2):
                    qt, kt = i // NKT, i % NKT
                    # stage 0: scores (+ per-qt Conv_K group start)
                    if i < NIT:
                        if kt == 0 and qt == 0:
                            pagg0 = psA.tile([C, TW], f32, tag="agg")
                            paggs[0] = pagg0
                            nc.tensor.matmul(pagg0[:], Wk_s[:],
                                             Xq[:, :TW],
                                             start=True, stop=False)
                        if kt == 2 and qt + 1 < NQT:
                            # issued just after agg[qt,31] so the single
                            # psA slot never deadlocks the in-order PE queue
                            pagg_n = psA.tile([C, TW], f32, tag="agg")
                            paggs[qt + 1] = pagg_n
                            nc.tensor.matmul(
                                pagg_n[:], Wk_s[:],
                                Xq[:, (qt + 1) * TW:(qt + 2) * TW],
                                start=True, stop=False)
                        psS = ps.tile([C, TW], f32, tag="ps")
                        nc.tensor.matmul(psS[:],
                                         Qc[:, kt * 128:(kt + 1) * 128],
                                         Xq[:, qt * TW:(qt + 1) * TW],
                                         start=True, stop=True)
                        Ei = work.tile([C, TW], bf16, tag="E")
                        Es[i] = Ei
                        nc.scalar.activation(Ei[:], psS[:], AF.Exp,
                                             scale=1.0 / RF)
                    # stage 1: block-sum matmul + divide (A = E/D in place)
                    if 1 <= i < NIT + 1:
                        j = i - 1
                        psD = ps.tile([C, TW], f32, tag="ps")
                        nc.tensor.matmul(psD[:], Bb[:], Es[j][:],
                                         start=True, stop=True)
                        eng = nc.gpsimd if (POOL_EVERY and
                                            j % POOL_EVERY == POOL_EVERY - 1) \
                            else nc.vector
                        eng.tensor_tensor(Es[j][:], Es[j][:], psD[:],
                                          op=OP.divide)
                    # stage 2: weighted-sum accumulate
                    if i >= 2:
                        j = i - 2
                        jqt, jkt = j // NKT, j % NKT
                        nc.tensor.matmul(paggs[jqt][:], Xnat[:, jkt, :],
                                         Es[j][:], start=False,
                                         stop=(jkt == NKT - 1))
                        Es[j] = None
                        if jkt == NKT - 1:
                            nc.vector.bn_stats(qs1[:, 2 * jqt, :],
                                               paggs[jqt][:, :512])
                            nc.vector.bn_stats(qs1[:, 2 * jqt + 1, :],
                                               paggs[jqt][:, 512:])
                            nc.gpsimd.tensor_copy(z1[:, jqt, :],
                                                  paggs[jqt][:])
                            paggs[jqt] = None

            # ---------------- epilogue (sharded) ----------------
            # partial BN_1 sums for this core's 2048 pixels
            mv1 = small.tile([C, 2], f32)
            nc.vector.bn_aggr(mv1[:], qs1[:])
            sums1 = small.tile([C, 2], f32)
            nc.vector.tensor_scalar_mul(sums1[:, 0:1], mv1[:, 0:1], float(QSH))
            m21 = small.tile([C, 1], f32)
            nc.vector.tensor_mul(m21[:], mv1[:, 0:1], mv1[:, 0:1])
            nc.vector.tensor_add(m21[:], m21[:], mv1[:, 1:2])
            nc.vector.tensor_scalar_mul(sums1[:, 1:2], m21[:], float(QSH))

            gst1 = gather8("st1", sums1, 2)
            # only a1 = g1*rsqrt(var+eps) matters: the spatial softmax is
            # invariant to the BN_1 shift (and to bt1)
            a1, _b1 = bn_affine(gst1, float(B * HWPIX), V[:, 2:3])

            # E1 = exp(a1 * z1) in one ACT pass; accum gives the shard sum
            E1 = big.tile([C, NQT, TW], bf16)
            esum = small.tile([C, 1], f32)
            nc.scalar.activation(E1[:].rearrange("p a b -> p (a b)"),
                                 z1[:].rearrange("p a b -> p (a b)"),
                                 AF.Exp, scale=a1[:], accum_out=esum[:])

            ges = gather8("es", esum, 1)
            # this core's spatial-softmax denominator: the two partials of
            # its own batch pair, selected by the per-core one-hot mask
            gsel = small.tile([C, 8], f32)
            nc.vector.tensor_mul(gsel[:], ges[:, :, 0], Msk[:])
            den = small.tile([C, 1], f32)
            nc.vector.tensor_reduce(den[:], gsel[:], axis=AX.X, op=OP.add)
            rb = small.tile([C, 1], f32)
            nc.vector.reciprocal(rb[:], den[:])

            # y = E1 * rb (in place), then CBL_O conv + batch stats
            zO = big.tile([C, NQT, TW], bf16)
            stO = small.tile([C, 2 * NQT, 6], f32)
            for t in range(NQT):
                with nc.allow_low_precision(reason="softmax weights bf16"):
                    nc.vector.tensor_scalar_mul(E1[:, t, :], E1[:, t, :],
                                                rb[:])
                pzo = ps.tile([C, TW], f32, tag="ps")
                nc.tensor.matmul(pzo[:], Wo_s[:], E1[:, t, :],
                                 start=True, stop=True)
                nc.vector.bn_stats(stO[:, 2 * t, :], pzo[:, :512])
                nc.vector.bn_stats(stO[:, 2 * t + 1, :], pzo[:, 512:])
                nc.gpsimd.tensor_copy(zO[:, t, :], pzo[:])

            mvO = small.tile([C, 2], f32)
            nc.vector.bn_aggr(mvO[:], stO[:])
            sumsO = small.tile([C, 2], f32)
            nc.vector.tensor_scalar_mul(sumsO[:, 0:1], mvO[:, 0:1], float(QSH))
            m2O = small.tile([C, 1], f32)
            nc.vector.tensor_mul(m2O[:], mvO[:, 0:1], mvO[:, 0:1])
            nc.vector.tensor_add(m2O[:], m2O[:], mvO[:, 1:2])
            nc.vector.tensor_scalar_mul(sumsO[:, 1:2], m2O[:], float(QSH))

            gstO = gather8("stO", sumsO, 2)
            aO, bO = bn_affine(gstO, float(B * HWPIX), V[:, 4:5], V[:, 5:6])

            OUT = big.tile([C, NQT, TW], f32)
            for t in range(NQT):
                tmp = tmp2p.tile([C, TW], f32, tag="tmp2")
                nc.scalar.activation(tmp[:], zO[:, t, :], AF.Identity,
                                     scale=aO[:], bias=bO[:])
                nc.vector.scalar_tensor_tensor(OUT[:, t, :], tmp[:], ALPHA,
                                               tmp[:], op0=OP.mult,
                                               op1=OP.max)
                eng = nc.sync if t % 2 == 0 else nc.scalar
                eng.dma_start(d_outT[:, t * TW:(t + 1) * TW], OUT[:, t, :])

    nc.compile()
    return nc


def _get_runner():
    if "runner" in _CACHE:
        return _CACHE["runner"]
    import jax
    import numpy as np
    from jax.sharding import Mesh, PartitionSpec
    from jax.experimental.shard_map import shard_map
    from concourse import mybir
    from concourse.bass2jax import (_bass_exec_p, install_neuronx_cc_hook,
                                    partition_id_tensor)

    nc = _build_program()
    install_neuronx_cc_hook()

    in_names, out_names, out_avals, zero_outs = [], [], [], []
    partition_name = nc.partition_id_tensor.name if nc.partition_id_tensor else None
    for alloc in nc.m.functions[0].allocations:
        if not isinstance(alloc, mybir.MemoryLocationSet):
            continue
        name = alloc.memorylocations[0].name
        if alloc.kind == "ExternalInput":
            if name != partition_name:
                in_names.append(name)
        elif alloc.kind == "ExternalOutput":
            shape = tuple(alloc.tensor_shape)
            dtype = mybir.dt.np(alloc.dtype)
            out_names.append(name)
            out_avals.append(jax.core.ShapedArray(shape, dtype))
            zero_outs.append(np.zeros(shape, dtype))
    n_params = len(in_names)
    n_outs = len(out_avals)
    all_in_names = list(in_names) + list(out_names)
    if partition_name is not None:
        all_in_names.append(partition_name)

    def _body(*args):
        operands = list(args)
        if partition_name is not None:
            operands.append(partition_id_tensor())
        outs = _bass_exec_p.bind(
            *operands,
            out_avals=tuple(out_avals),
            in_names=tuple(all_in_names),
            out_names=tuple(out_names),
            lowering_input_output_aliases=(),
            sim_require_finite=True,
            sim_require_nnan=True,
            nc=nc,
        )
        return tuple(outs)

    donate = tuple(range(n_params, n_params + n_outs))
    try:
        devices = jax.devices("axon")[:N_CORES]
    except RuntimeError:
        devices = jax.devices()[:N_CORES]
    mesh = Mesh(np.asarray(devices), ("core",))
    in_specs = (PartitionSpec("core"),) * (n_params + n_outs)
    out_specs = (PartitionSpec("core"),) * n_outs
    sharded = jax.jit(
        shard_map(_body, mesh=mesh, in_specs=in_specs, out_specs=out_specs,
                  check_rep=False),
        donate_argnums=donate, keep_unused=True)

    def run(in_maps):
        per_core = [[np.asarray(m[name]) for name in in_names] for m in in_maps]
        concat_in = [np.concatenate([per_core[c][i] for c in range(N_CORES)],
                                    axis=0) for i in range(n_params)]
        concat_zeros = [np.zeros((N_CORES * z.shape[0], *z.shape[1:]), z.dtype)
                        for z in zero_outs]
        out_arrs = jax.block_until_ready(sharded(*concat_in, *concat_zeros))
        return [
            {name: np.asarray(out_arrs[i]).reshape(N_CORES, *out_avals[i].shape)[c]
             for i, name in enumerate(out_names)}
            for c in range(N_CORES)
        ]

    _CACHE["runner"] = run
    return run


def _make_blockmap():
    bm = np.zeros((C, C), np.float32)
    idx = np.arange(C)
    bm[(idx[:, None] // 64) == (idx[None, :] // 64)] = 1.0
    return bm


def kernel(x, Wq, bq, gq, btq, Wk, bk, g1, bt1, Wo, bo, go, bto):
    """Full inputs -> full output. Conv biases cancel inside training-mode
    BN (the mean subtraction removes any per-channel constant), so bq/bk/bo
    never enter the device program."""
    import ml_dtypes
    bf16 = ml_dtypes.bfloat16

    x = np.asarray(x, np.float32)
    run = _get_runner()

    wq9 = np.ascontiguousarray(
        np.asarray(Wq, np.float32).reshape(9, C, C)).astype(bf16)
    wk = np.ascontiguousarray(
        np.asarray(Wk, np.float32).reshape(C, C)).astype(bf16)
    wo = np.ascontiguousarray(
        np.asarray(Wo, np.float32).reshape(C, C)).astype(bf16)
    vecs = np.ascontiguousarray(np.stack([
        np.asarray(v, np.float32) for v in (gq, btq, g1, bt1, go, bto)]))
    bm = _make_blockmap().astype(bf16)

    # block-major key permutation: index (n,m,p,q) -> pixel (8n+p, 8m+q)
    perm = np.arange(HWPIX).reshape(8, 8, 8, 8).transpose(0, 2, 1, 3).reshape(-1)

    in_maps = []
    for core in range(N_CORES):
        b, h = core // 2, core % 2
        xb = np.ascontiguousarray(x[b].reshape(HWPIX, C))
        xbT = xb.T  # [C, HWPIX]
        xqT = np.ascontiguousarray(xbT[:, h * QSH:(h + 1) * QSH]).astype(bf16)
        xpadT = np.zeros((C, H + 2, W + 2), np.float32)
        xpadT[:, 1:H + 1, 1:W + 1] = xbT.reshape(C, H, W)
        msk = np.zeros((8, C), np.float32)
        msk[2 * b] = 1.0
        msk[2 * b + 1] = 1.0
        in_maps.append({
            "xnat": np.ascontiguousarray(xb[perm]).astype(bf16),
            "xqT": xqT,
            "xpadT": np.ascontiguousarray(
                xpadT.reshape(C, PADN)).astype(bf16),
            "wq9": wq9, "wk": wk, "wo": wo, "vecs": vecs, "msk": msk,
            "bm": bm,
        })

    res = run(in_maps)
    full = np.empty((B, HWPIX, C), np.float32)
    for core in range(N_CORES):
        b, h = core // 2, core % 2
        full[b, h * QSH:(h + 1) * QSH, :] = res[core]["outT"].T
    return full.reshape(B, H, W, C)


# revision 8
# speedup vs baseline: 2.3996x; 1.0212x over previous
"""Trainium2 Bass kernel for nn_GroupAttentionLayer (sparse block attention).

Strategy (8 NeuronCores, SPMD): core i handles batch b=i//2, query-pixel
half h=i%2 (2048 query pixels each). All heavy tensors are bf16 (PSUM
accumulation stays fp32), attention tiles are 1024 queries wide, and the
epilogue is fully SHARDED (the 1x1 conv + BN/softmax reductions are
pointwise, so no core ever needs the full tensor):

    scores^T[k,q] = Qc[:,k].T @ Xq[:,q]      (PE, contract channels, bf16)
    E = exp(scores/8)                        (ACT, bf16 out)
    D_bcast = blockmap.T @ E                 (PE, per-64-block sums)
    A = E / D_bcast                          (DVE tensor_tensor divide,
                                              every POOL_EVERY-th on Pool)
    agg^T[c,q] += x_blk[k,:].T @ A           (PE, PSUM accum, Conv_K folded)

The PE instruction stream is software-pipelined (scores issued 2 iters
ahead, blockmap 1 ahead) so the in-order engine queues never stall on
cross-engine dependencies.

Four tiny AllGathers (cheaper than AllReduce under the collective cost
model) carry the cross-core reductions: BN_Q batch stats, BN_1 batch
stats, per-batch spatial-softmax exp sums, BN_O batch stats. BN shifts
that feed the spatial softmax cancel algebraically (softmax is
shift-invariant) and conv biases cancel inside training-mode BN, so
neither is computed. A per-core one-hot mask input selects this core's
batch pair out of the gathered exp sums.

Host side: shards/transposes/bf16-converts inputs with numpy, assembles
the output from the 8 per-core channel-major shards.
"""

import numpy as np

B, H, W, C = 4, 64, 64, 128
RF = 8
EPS = 1e-3
ALPHA = 0.1
N_CORES = 8
HWPIX = H * W             # 4096 pixels per batch
QSH = HWPIX * B // N_CORES  # 2048 query pixels per core
PW = W + 2                # 66, padded row width
PADN = PW * (H + 2)       # 4356 padded columns
NKT = HWPIX // 128        # 32 key tiles per batch
TW = 1024                 # attention tile width (queries)
NQT = QSH // TW           # 2 query tiles per core
NCC = HWPIX // TW         # 4 conv chunks (2 block-rows each)
NIT = NQT * NKT           # 64 inner iterations

# every POOL_EVERY-th divide runs on Pool (gpsimd), the rest on DVE.
# 0 disables the Pool share (DVE has slack; fewer semaphores)
POOL_EVERY = 0

_CACHE = {}


def _build_program():
    import concourse.bacc as bacc
    import concourse.tile as tile
    from concourse import mybir

    f32 = mybir.dt.float32
    bf16 = mybir.dt.bfloat16
    AF = mybir.ActivationFunctionType
    OP = mybir.AluOpType
    AX = mybir.AxisListType

    nc = bacc.Bacc("TRN2", target_bir_lowering=False, debug=False,
                   enable_asserts=True, num_devices=N_CORES)

    # per-core inputs (bf16 heavy data, fp32 vectors)
    d_xnat = nc.dram_tensor("xnat", [HWPIX, C], bf16, kind="ExternalInput").ap()
    d_xqT = nc.dram_tensor("xqT", [C, QSH], bf16, kind="ExternalInput").ap()
    d_xpadT = nc.dram_tensor("xpadT", [C, PADN], bf16, kind="ExternalInput").ap()
    d_wq9 = nc.dram_tensor("wq9", [9, C, C], bf16, kind="ExternalInput").ap()
    d_wk = nc.dram_tensor("wk", [C, C], bf16, kind="ExternalInput").ap()
    d_wo = nc.dram_tensor("wo", [C, C], bf16, kind="ExternalInput").ap()
    d_vecs = nc.dram_tensor("vecs", [6, C], f32, kind="ExternalInput").ap()
    d_msk = nc.dram_tensor("msk", [8, C], f32, kind="ExternalInput").ap()
    d_bm = nc.dram_tensor("bm", [C, C], bf16, kind="ExternalInput").ap()
    # output: this core's channel-major shard
    d_outT = nc.dram_tensor("outT", [C, QSH], f32, kind="ExternalOutput").ap()

    with tile.TileContext(nc) as tc:
        with tc.tile_pool(name="const", bufs=1) as const, \
             tc.tile_pool(name="big", bufs=1) as big, \
             tc.tile_pool(name="work", bufs=3) as work, \
             tc.tile_pool(name="tmp2", bufs=2) as tmp2p, \
             tc.tile_pool(name="small", bufs=2) as small, \
             tc.tile_pool(name="ps", bufs=3, space="PSUM") as ps, \
             tc.tile_pool(name="psA", bufs=1, space="PSUM") as psA, \
             tc.tile_pool(name="dram", bufs=1, space="DRAM") as dram:

            # ---------------- loads ----------------
            # conv inputs (Wq + Xpad row-chunks) first on their queues so the
            # conv can start ~3us in; everything else behind them
            Wq_s = const.tile([C, 9, C], bf16)
            nc.scalar.dma_start(
                Wq_s[:], d_wq9.rearrange("t ci co -> ci t co"))
            Xpad = big.tile([C, PADN], bf16)
            Xpr = Xpad[:].rearrange("p (r c) -> p r c", r=H + 2)
            d_xpr = d_xpadT.rearrange("p (r c) -> p r c", r=H + 2)
            for rr in range(4):
                r0, r1 = [(0, 18), (18, 34), (34, 50), (50, 66)][rr]
                nc.sync.dma_start(Xpr[:, r0:r1, :], d_xpr[:, r0:r1, :])
            Xq = big.tile([C, QSH], bf16)
            nc.sync.dma_start(Xq[:], d_xqT[:])
            Xnat = big.tile([128, NKT, C], bf16)
            nc.scalar.dma_start(
                Xnat[:], d_xnat.rearrange("(t p) c -> p t c", p=128))
            Wk_s = const.tile([C, C], bf16)
            nc.sync.dma_start(Wk_s[:], d_wk[:])
            Wo_s = const.tile([C, C], bf16)
            nc.sync.dma_start(Wo_s[:], d_wo[:])
            V = const.tile([C, 6], f32)
            nc.scalar.dma_start(V[:], d_vecs.rearrange("v c -> c v"))
            Msk = const.tile([C, 8], f32)
            nc.scalar.dma_start(Msk[:], d_msk.rearrange("v c -> c v"))
            Bb = const.tile([C, C], bf16)
            nc.sync.dma_start(Bb[:], d_bm[:])
            eps_t = const.tile([C, 1], f32)
            nc.vector.memset(eps_t[:], EPS)
            # dummy ln: forces the ln/exp activation table load here, off the
            # post-collective critical path
            scratch1 = const.tile([C, 1], f32)
            nc.scalar.activation(scratch1[:], eps_t[:], AF.Ln)
            Wz = const.tile([C, TW], bf16)
            nc.vector.memset(Wz[:], 0.0)

            def warm(n):
                # dependency-free matmuls that keep the PE pstate at full
                # clock through windows where real work is blocked
                for _ in range(n):
                    wp = ps.tile([C, TW], f32, tag="ps")
                    nc.tensor.matmul(wp[:], Wz[:, :C], Wz[:],
                                     start=True, stop=True)

            warm(10)

            Xpv = Xpad[:].rearrange("p (r c) -> p r c", r=H + 2)

            # ---------------- CBL_Q: conv3x3 + batch stats ----------------
            # Conv output in BLOCK-MAJOR key order: chunk t covers block rows
            # n=2t,2t+1; column n*512 + m*64 + p*8 + q is pixel (8n+p, 8m+q).
            Zq = big.tile([C, NCC, TW], bf16)
            qstats = small.tile([C, 2 * NCC, 6], f32)
            for t in range(NCC):
                pq = ps.tile([C, TW], f32, tag="ps")
                for tap in range(9):
                    dh, dw = tap // 3 - 1, tap % 3 - 1
                    rhs = Xpv[:, t * 16 + 1 + dh: t * 16 + 17 + dh,
                              1 + dw: 65 + dw].rearrange(
                                  "c (n p) (m q) -> c n m p q", n=2, m=8)
                    nc.tensor.matmul(pq[:], Wq_s[:, tap, :], rhs,
                                     start=(tap == 0), stop=(tap == 8))
                nc.vector.bn_stats(qstats[:, 2 * t, :], pq[:, :512])
                nc.vector.bn_stats(qstats[:, 2 * t + 1, :], pq[:, 512:])
                nc.gpsimd.tensor_copy(Zq[:, t, :], pq[:])

            qmv = small.tile([C, 2], f32)
            nc.vector.bn_aggr(qmv[:], qstats[:])
            # partial sums for the cross-core stats:
            #   sums[:,0] = mean * 4096 ; sums[:,1] = (var + mean^2) * 4096
            sums = small.tile([C, 2], f32)
            nc.vector.tensor_scalar_mul(sums[:, 0:1], qmv[:, 0:1], float(HWPIX))
            m2 = small.tile([C, 1], f32)
            nc.vector.tensor_mul(m2[:], qmv[:, 0:1], qmv[:, 0:1])
            nc.vector.tensor_add(m2[:], m2[:], qmv[:, 1:2])
            nc.vector.tensor_scalar_mul(sums[:, 1:2], m2[:], float(HWPIX))

            grp = [list(range(N_CORES))]

            def gather8(name, src, width, warm_n=0):
                """AllGather a [C,width] fp32 tile -> [C,8,width] on-chip."""
                t_in = dram.tile([C, width], f32, tag=f"{name}_in")
                t_out = dram.tile([N_CORES * C, width], f32,
                                  addr_space="Shared", tag=f"{name}_out")
                nc.sync.dma_start(t_in[:], src[:])
                nc.gpsimd.collective_compute(
                    "AllGather", OP.bypass, replica_groups=grp,
                    ins=[t_in.opt()], outs=[t_out.opt()])
                if warm_n:
                    warm(warm_n)
                g = small.tile([C, 8, width], f32, tag=f"{name}_g")
                nc.sync.dma_start(
                    g[:], t_out[:].rearrange("(r p) s -> p r s", r=N_CORES))
                return g

            def bn_affine(gst8, tot, gamma, beta=None):
                """gst8: [C,8,2] gathered (sum, sumsq) partials. Returns
                (a, b): a = gamma*rsqrt(var+eps), b = beta - a*mean (b=None
                when beta is None -- the shift cancels downstream)."""
                gsum = small.tile([C, 2], f32, tag="gsum")
                nc.vector.tensor_reduce(
                    gsum[:], gst8[:].rearrange("c r s -> c s r"),
                    axis=AX.X, op=OP.add)
                sc = small.tile([C, 2], f32, tag="scmom")
                nc.vector.tensor_scalar_mul(sc[:], gsum[:], 1.0 / tot)
                negvar = small.tile([C, 1], f32, tag="negvar")
                nc.vector.scalar_tensor_tensor(negvar[:], sc[:, 0:1],
                                               sc[:, 0:1], sc[:, 1:2],
                                               op0=OP.mult, op1=OP.subtract)
                # rsqrt(var+eps) = exp(-0.5*ln(var+eps)); ln/exp share one
                # ACT table set, so no LoadActFuncSet in this chain
                lnv = small.tile([C, 1], f32, tag="lnv")
                nc.scalar.activation(lnv[:], negvar[:], AF.Ln,
                                     scale=-1.0, bias=eps_t[:])
                rstd = small.tile([C, 1], f32, tag="rstd")
                nc.scalar.activation(rstd[:], lnv[:], AF.Exp, scale=-0.5)
                a = small.tile([C, 1], f32, tag="acoef")
                nc.vector.tensor_mul(a[:], rstd[:], gamma)
                if beta is None:
                    return a, None
                b = small.tile([C, 1], f32, tag="bcoef")
                nc.vector.tensor_scalar(b[:], sc[:, 0:1], a[:], -1.0,
                                        op0=OP.mult, op1=OP.mult)
                nc.vector.tensor_add(b[:], b[:], beta)
                return a, b

            gstq = gather8("stq", sums, 2, warm_n=56)
            aq, bq = bn_affine(gstq, float(HWPIX * N_CORES),
                               V[:, 0:1], V[:, 1:2])

            # q = leaky(aq*z + bq); Zq is already block-major. Chunk 0 is
            # emitted here; chunks 1-3 are interleaved into the main loop
            # (issued a few iterations before their keys are needed) so the
            # in-order ACT queue never blocks exp_0 behind them.
            Qc = big.tile([C, HWPIX], bf16)
            Qv = Qc[:].rearrange("p (t f) -> p t f", f=TW)

            def affine_chunk(t):
                tmp = tmp2p.tile([C, TW], f32, tag="tmp2")
                nc.scalar.activation(tmp[:], Zq[:, t, :], AF.Identity,
                                     scale=aq[:], bias=bq[:])
                nc.vector.scalar_tensor_tensor(Qv[:, t, :], tmp[:], ALPHA,
                                               tmp[:], op0=OP.mult,
                                               op1=OP.max)

            affine_chunk(0)

            # ---------------- attention main loop (software-pipelined) ----
            z1 = big.tile([C, NQT, TW], bf16)
            qs1 = small.tile([C, 2 * NQT, 6], f32)
            paggs = [None] * NQT
            Es = [None] * (NIT + 2)
            with nc.allow_low_precision(reason="attn weights in bf16"):
                for i in range(NIT + 2):
                    qt, kt = i // NKT, i % NKT
                    if qt == 0 and kt in (5, 13, 21):
                        affine_chunk(kt // 8 + 1)
                    # stage 0: scores (+ per-qt Conv_K group start)
                    if i < NIT:
                        if kt == 0 and qt == 0:
                            pagg0 = psA.tile([C, TW], f32, tag="agg")
                            paggs[0] = pagg0
                            nc.tensor.matmul(pagg0[:], Wk_s[:],
                                             Xq[:, :TW],
                                             start=True, stop=False)
                        if kt == 2 and qt + 1 < NQT:
                            # issued just after agg[qt,31] so the single
                            # psA slot never deadlocks the in-order PE queue
                            pagg_n = psA.tile([C, TW], f32, tag="agg")
                            paggs[qt + 1] = pagg_n
                            nc.tensor.matmul(
                                pagg_n[:], Wk_s[:],
                                Xq[:, (qt + 1) * TW:(qt + 2) * TW],
                                start=True, stop=False)
                        psS = ps.tile([C, TW], f32, tag="ps")
                        nc.tensor.matmul(psS[:],
                                         Qc[:, kt * 128:(kt + 1) * 128],
                                         Xq[:, qt * TW:(qt + 1) * TW],
                                         start=True, stop=True)
                        Ei = work.tile([C, TW], bf16, tag="E")
                        Es[i] = Ei
                        nc.scalar.activation(Ei[:], psS[:], AF.Exp,
                                             scale=1.0 / RF)
                    # stage 1: block-sum matmul + divide (A = E/D in place)
                    if 1 <= i < NIT + 1:
                        j = i - 1
                        psD = ps.tile([C, TW], f32, tag="ps")
                        nc.tensor.matmul(psD[:], Bb[:], Es[j][:],
                                         start=True, stop=True)
                        eng = nc.gpsimd if (POOL_EVERY and
                                            j % POOL_EVERY == POOL_EVERY - 1) \
                            else nc.vector
                        eng.tensor_tensor(Es[j][:], Es[j][:], psD[:],
                                          op=OP.divide)
                    # stage 2: weighted-sum accumulate
                    if i >= 2:
                        j = i - 2
                        jqt, jkt = j // NKT, j % NKT
                        nc.tensor.matmul(paggs[jqt][:], Xnat[:, jkt, :],
                                         Es[j][:], start=False,
                                         stop=(jkt == NKT - 1))
                        Es[j] = None
                        if jkt == NKT - 1:
                            nc.vector.bn_stats(qs1[:, 2 * jqt, :],
                                               paggs[jqt][:, :512])
                            nc.vector.bn_stats(qs1[:, 2 * jqt + 1, :],
                                               paggs[jqt][:, 512:])
                            nc.gpsimd.tensor_copy(z1[:, jqt, :],
                                                  paggs[jqt][:])
                            paggs[jqt] = None

            # ---------------- epilogue (sharded) ----------------
            # partial BN_1 sums for this core's 2048 pixels
            mv1 = small.tile([C, 2], f32)
            nc.vector.bn_aggr(mv1[:], qs1[:])
            sums1 = small.tile([C, 2], f32)
            nc.vector.tensor_scalar_mul(sums1[:, 0:1], mv1[:, 0:1], float(QSH))
            m21 = small.tile([C, 1], f32)
            nc.vector.tensor_mul(m21[:], mv1[:, 0:1], mv1[:, 0:1])
            nc.vector.tensor_add(m21[:], m21[:], mv1[:, 1:2])
            nc.vector.tensor_scalar_mul(sums1[:, 1:2], m21[:], float(QSH))

            gst1 = gather8("st1", sums1, 2)
            # only a1 = g1*rsqrt(var+eps) matters: the spatial softmax is
            # invariant to the BN_1 shift (and to bt1)
            a1, _b1 = bn_affine(gst1, float(B * HWPIX), V[:, 2:3])

            # E1 = exp(a1 * z1) in one ACT pass; accum gives the shard sum
            E1 = big.tile([C, NQT, TW], bf16)
            esum = small.tile([C, 1], f32)
            nc.scalar.activation(E1[:].rearrange("p a b -> p (a b)"),
                                 z1[:].rearrange("p a b -> p (a b)"),
                                 AF.Exp, scale=a1[:], accum_out=esum[:])

            ges = gather8("es", esum, 1)
            # this core's spatial-softmax denominator: the two partials of
            # its own batch pair, selected by the per-core one-hot mask
            gsel = small.tile([C, 8], f32)
            nc.vector.tensor_mul(gsel[:], ges[:, :, 0], Msk[:])
            den = small.tile([C, 1], f32)
            nc.vector.tensor_reduce(den[:], gsel[:], axis=AX.X, op=OP.add)
            rb = small.tile([C, 1], f32)
            nc.vector.reciprocal(rb[:], den[:])

            # y = E1 * rb (in place), then CBL_O conv + batch stats
            zO = big.tile([C, NQT, TW], bf16)
            stO = small.tile([C, 2 * NQT, 6], f32)
            for t in range(NQT):
                with nc.allow_low_precision(reason="softmax weights bf16"):
                    nc.vector.tensor_scalar_mul(E1[:, t, :], E1[:, t, :],
                                                rb[:])
                pzo = ps.tile([C, TW], f32, tag="ps")
                nc.tensor.matmul(pzo[:], Wo_s[:], E1[:, t, :],
                                 start=True, stop=True)
                nc.vector.bn_stats(stO[:, 2 * t, :], pzo[:, :512])
                nc.vector.bn_stats(stO[:, 2 * t + 1, :], pzo[:, 512:])
                nc.gpsimd.tensor_copy(zO[:, t, :], pzo[:])

            mvO = small.tile([C, 2], f32)
            nc.vector.bn_aggr(mvO[:], stO[:])
            sumsO = small.tile([C, 2], f32)
            nc.vector.tensor_scalar_mul(sumsO[:, 0:1], mvO[:, 0:1], float(QSH))
            m2O = small.tile([C, 1], f32)
            nc.vector.tensor_mul(m2O[:], mvO[:, 0:1], mvO[:, 0:1])
            nc.vector.tensor_add(m2O[:], m2O[:], mvO[:, 1:2])
            nc.vector.tensor_scalar_mul(sumsO[:, 1:2], m2O[:], float(QSH))

            gstO = gather8("stO", sumsO, 2)
            aO, bO = bn_affine(gstO, float(B * HWPIX), V[:, 4:5], V[:, 5:6])

            OUT = big.tile([C, NQT, TW], f32)
            for t in range(NQT):
                tmp = tmp2p.tile([C, TW], f32, tag="tmp2")
                nc.scalar.activation(tmp[:], zO[:, t, :], AF.Identity,
                                     scale=aO[:], bias=bO[:])
                nc.vector.scalar_tensor_tensor(OUT[:, t, :], tmp[:], ALPHA,
                                               tmp[:], op0=OP.mult,
                                               op1=OP.max)
                eng = nc.sync if t % 2 == 0 else nc.scalar
                eng.dma_start(d_outT[:, t * TW:(t + 1) * TW], OUT[:, t, :])

    nc.compile()
    return nc


def _get_runner():
    if "runner" in _CACHE:
        return _CACHE["runner"]
    import jax
    import numpy as np
    from jax.sharding import Mesh, PartitionSpec
    from jax.experimental.shard_map import shard_map
    from concourse import mybir
    from concourse.bass2jax import (_bass_exec_p, install_neuronx_cc_hook,
                                    partition_id_tensor)

    nc = _build_program()
    install_neuronx_cc_hook()

    in_names, out_names, out_avals, zero_outs = [], [], [], []
    partition_name = nc.partition_id_tensor.name if nc.partition_id_tensor else None
    for alloc in nc.m.functions[0].allocations:
        if not isinstance(alloc, mybir.MemoryLocationSet):
            continue
        name = alloc.memorylocations[0].name
        if alloc.kind == "ExternalInput":
            if name != partition_name:
                in_names.append(name)
        elif alloc.kind == "ExternalOutput":
            shape = tuple(alloc.tensor_shape)
            dtype = mybir.dt.np(alloc.dtype)
            out_names.append(name)
            out_avals.append(jax.core.ShapedArray(shape, dtype))
            zero_outs.append(np.zeros(shape, dtype))
    n_params = len(in_names)
    n_outs = len(out_avals)
    all_in_names = list(in_names) + list(out_names)
    if partition_name is not None:
        all_in_names.append(partition_name)

    def _body(*args):
        operands = list(args)
        if partition_name is not None:
            operands.append(partition_id_tensor())
        outs = _bass_exec_p.bind(
            *operands,
            out_avals=tuple(out_avals),
            in_names=tuple(all_in_names),
            out_names=tuple(out_names),
            lowering_input_output_aliases=(),
            sim_require_finite=True,
            sim_require_nnan=True,
            nc=nc,
        )
        return tuple(outs)

    donate = tuple(range(n_params, n_params + n_outs))
    try:
        devices = jax.devices("axon")[:N_CORES]
    except RuntimeError:
        devices = jax.devices()[:N_CORES]
    mesh = Mesh(np.asarray(devices), ("core",))
    in_specs = (PartitionSpec("core"),) * (n_params + n_outs)
    out_specs = (PartitionSpec("core"),) * n_outs
    sharded = jax.jit(
        shard_map(_body, mesh=mesh, in_specs=in_specs, out_specs=out_specs,
                  check_rep=False),
        donate_argnums=donate, keep_unused=True)

    def run(in_maps):
        per_core = [[np.asarray(m[name]) for name in in_names] for m in in_maps]
        concat_in = [np.concatenate([per_core[c][i] for c in range(N_CORES)],
                                    axis=0) for i in range(n_params)]
        concat_zeros = [np.zeros((N_CORES * z.shape[0], *z.shape[1:]), z.dtype)
                        for z in zero_outs]
        out_arrs = jax.block_until_ready(sharded(*concat_in, *concat_zeros))
        return [
            {name: np.asarray(out_arrs[i]).reshape(N_CORES, *out_avals[i].shape)[c]
             for i, name in enumerate(out_names)}
            for c in range(N_CORES)
        ]

    _CACHE["runner"] = run
    return run


def _make_blockmap():
    bm = np.zeros((C, C), np.float32)
    idx = np.arange(C)
    bm[(idx[:, None] // 64) == (idx[None, :] // 64)] = 1.0
    return bm


def kernel(x, Wq, bq, gq, btq, Wk, bk, g1, bt1, Wo, bo, go, bto):
    """Full inputs -> full output. Conv biases cancel inside training-mode
    BN (the mean subtraction removes any per-channel constant), so bq/bk/bo
    never enter the device program."""
    import ml_dtypes
    bf16 = ml_dtypes.bfloat16

    x = np.asarray(x, np.float32)
    run = _get_runner()

    wq9 = np.ascontiguousarray(
        np.asarray(Wq, np.float32).reshape(9, C, C)).astype(bf16)
    wk = np.ascontiguousarray(
        np.asarray(Wk, np.float32).reshape(C, C)).astype(bf16)
    wo = np.ascontiguousarray(
        np.asarray(Wo, np.float32).reshape(C, C)).astype(bf16)
    vecs = np.ascontiguousarray(np.stack([
        np.asarray(v, np.float32) for v in (gq, btq, g1, bt1, go, bto)]))
    bm = _make_blockmap().astype(bf16)

    # block-major key permutation: index (n,m,p,q) -> pixel (8n+p, 8m+q)
    perm = np.arange(HWPIX).reshape(8, 8, 8, 8).transpose(0, 2, 1, 3).reshape(-1)

    in_maps = []
    for core in range(N_CORES):
        b, h = core // 2, core % 2
        xb = np.ascontiguousarray(x[b].reshape(HWPIX, C))
        xbT = xb.T  # [C, HWPIX]
        xqT = np.ascontiguousarray(xbT[:, h * QSH:(h + 1) * QSH]).astype(bf16)
        xpadT = np.zeros((C, H + 2, W + 2), np.float32)
        xpadT[:, 1:H + 1, 1:W + 1] = xbT.reshape(C, H, W)
        msk = np.zeros((8, C), np.float32)
        msk[2 * b] = 1.0
        msk[2 * b + 1] = 1.0
        in_maps.append({
            "xnat": np.ascontiguousarray(xb[perm]).astype(bf16),
            "xqT": xqT,
            "xpadT": np.ascontiguousarray(
                xpadT.reshape(C, PADN)).astype(bf16),
            "wq9": wq9, "wk": wk, "wo": wo, "vecs": vecs, "msk": msk,
            "bm": bm,
        })

    res = run(in_maps)
    full = np.empty((B, HWPIX, C), np.float32)
    for core in range(N_CORES):
        b, h = core // 2, core % 2
        full[b, h * QSH:(h + 1) * QSH, :] = res[core]["outT"].T
    return full.reshape(B, H, W, C)


# revision 9
# speedup vs baseline: 2.4386x; 1.0163x over previous
"""Trainium2 Bass kernel for nn_GroupAttentionLayer (sparse block attention).

Strategy (8 NeuronCores, SPMD): core i handles batch b=i//2, query-pixel
half h=i%2 (2048 query pixels each). All heavy tensors are bf16 (PSUM
accumulation stays fp32), attention tiles are 1024 queries wide, and the
epilogue is fully SHARDED (the 1x1 conv + BN/softmax reductions are
pointwise, so no core ever needs the full tensor):

    scores^T[k,q] = Qc[:,k].T @ Xq[:,q]      (PE, contract channels, bf16)
    E = exp(scores/8)                        (ACT, bf16 out)
    D_bcast = blockmap.T @ E                 (PE, per-64-block sums)
    A = E / D_bcast                          (DVE tensor_tensor divide,
                                              every POOL_EVERY-th on Pool)
    agg^T[c,q] += x_blk[k,:].T @ A           (PE, PSUM accum, Conv_K folded)

The PE instruction stream is software-pipelined (scores issued 2 iters
ahead, blockmap 1 ahead) so the in-order engine queues never stall on
cross-engine dependencies.

Four tiny AllGathers (cheaper than AllReduce under the collective cost
model) carry the cross-core reductions: BN_Q batch stats, BN_1 batch
stats, per-batch spatial-softmax exp sums, BN_O batch stats. BN shifts
that feed the spatial softmax cancel algebraically (softmax is
shift-invariant) and conv biases cancel inside training-mode BN, so
neither is computed. A per-core one-hot mask input selects this core's
batch pair out of the gathered exp sums.

Host side: shards/transposes/bf16-converts inputs with numpy, assembles
the output from the 8 per-core channel-major shards.
"""

import numpy as np

B, H, W, C = 4, 64, 64, 128
RF = 8
EPS = 1e-3
ALPHA = 0.1
N_CORES = 8
HWPIX = H * W             # 4096 pixels per batch
QSH = HWPIX * B // N_CORES  # 2048 query pixels per core
PW = W + 2                # 66, padded row width
PADN = PW * (H + 2)       # 4356 padded columns
NKT = HWPIX // 128        # 32 key tiles per batch
TW = 1024                 # attention tile width (queries)
NQT = QSH // TW           # 2 query tiles per core
NCC = HWPIX // TW         # 4 conv chunks (2 block-rows each)
NIT = NQT * NKT           # 64 inner iterations

# every POOL_EVERY-th divide runs on Pool (gpsimd), the rest on DVE.
# 0 disables the Pool share (DVE has slack; fewer semaphores)
POOL_EVERY = 0

_CACHE = {}


def _build_program():
    import concourse.bacc as bacc
    import concourse.tile as tile
    from concourse import mybir

    f32 = mybir.dt.float32
    bf16 = mybir.dt.bfloat16
    AF = mybir.ActivationFunctionType
    OP = mybir.AluOpType
    AX = mybir.AxisListType

    nc = bacc.Bacc("TRN2", target_bir_lowering=False, debug=False,
                   enable_asserts=True, num_devices=N_CORES)

    # per-core inputs (bf16 heavy data, fp32 vectors)
    d_xnat = nc.dram_tensor("xnat", [HWPIX, C], bf16, kind="ExternalInput").ap()
    d_xqT = nc.dram_tensor("xqT", [C, QSH], bf16, kind="ExternalInput").ap()
    d_xpadT = nc.dram_tensor("xpadT", [C, PADN], bf16, kind="ExternalInput").ap()
    d_wq9 = nc.dram_tensor("wq9", [9, C, C], bf16, kind="ExternalInput").ap()
    d_wk = nc.dram_tensor("wk", [C, C], bf16, kind="ExternalInput").ap()
    d_wo = nc.dram_tensor("wo", [C, C], bf16, kind="ExternalInput").ap()
    d_vecs = nc.dram_tensor("vecs", [6, C], f32, kind="ExternalInput").ap()
    d_msk = nc.dram_tensor("msk", [8, C], f32, kind="ExternalInput").ap()
    d_bm = nc.dram_tensor("bm", [C, C], bf16, kind="ExternalInput").ap()
    # output: this core's channel-major shard
    d_outT = nc.dram_tensor("outT", [C, QSH], f32, kind="ExternalOutput").ap()

    with tile.TileContext(nc) as tc:
        with tc.tile_pool(name="const", bufs=1) as const, \
             tc.tile_pool(name="big", bufs=1) as big, \
             tc.tile_pool(name="work", bufs=3) as work, \
             tc.tile_pool(name="tmp2", bufs=2) as tmp2p, \
             tc.tile_pool(name="small", bufs=2) as small, \
             tc.tile_pool(name="ps", bufs=3, space="PSUM") as ps, \
             tc.tile_pool(name="psA", bufs=1, space="PSUM") as psA, \
             tc.tile_pool(name="dram", bufs=1, space="DRAM") as dram:

            # ---------------- loads ----------------
            # conv inputs (Wq + Xpad row-chunks) first on their queues so the
            # conv can start ~3us in; everything else behind them
            Wq_s = const.tile([C, 9, C], bf16)
            nc.scalar.dma_start(
                Wq_s[:], d_wq9.rearrange("t ci co -> ci t co"))
            Xpad = big.tile([C, PADN], bf16)
            Xpr = Xpad[:].rearrange("p (r c) -> p r c", r=H + 2)
            d_xpr = d_xpadT.rearrange("p (r c) -> p r c", r=H + 2)
            for rr in range(4):
                r0, r1 = [(0, 18), (18, 34), (34, 50), (50, 66)][rr]
                nc.sync.dma_start(Xpr[:, r0:r1, :], d_xpr[:, r0:r1, :])
            Xq = big.tile([C, QSH], bf16)
            nc.sync.dma_start(Xq[:], d_xqT[:])
            Xnat = big.tile([128, NKT, C], bf16)
            nc.scalar.dma_start(
                Xnat[:], d_xnat.rearrange("(t p) c -> p t c", p=128))
            Wk_s = const.tile([C, C], bf16)
            nc.sync.dma_start(Wk_s[:], d_wk[:])
            Wo_s = const.tile([C, C], bf16)
            nc.sync.dma_start(Wo_s[:], d_wo[:])
            V = const.tile([C, 6], f32)
            nc.scalar.dma_start(V[:], d_vecs.rearrange("v c -> c v"))
            Msk = const.tile([C, 8], f32)
            nc.scalar.dma_start(Msk[:], d_msk.rearrange("v c -> c v"))
            Bb = const.tile([C, C], bf16)
            nc.sync.dma_start(Bb[:], d_bm[:])
            eps_t = const.tile([C, 1], f32)
            nc.vector.memset(eps_t[:], EPS)
            # load the ln+exp activation table once, up front: every ACT
            # function used below (Exp, Ln, Identity) lives in this set, so
            # the compiler pass inserts no further mid-chain table reloads
            from concourse.hw_specs import get_activation_tables
            _tabs = list(get_activation_tables(nc.m.arch))
            _set_id = _tabs.index("natural_log_exp_and_others")
            nc.scalar.add_instruction(mybir.InstLoadActFuncSet(
                name=nc.get_next_instruction_name(), ins=[], outs=[],
                act_func_set_id=_set_id))
            Wz = const.tile([C, TW], bf16)
            nc.vector.memset(Wz[:], 0.0)

            def warm(n):
                # dependency-free matmuls that keep the PE pstate at full
                # clock through windows where real work is blocked
                for _ in range(n):
                    wp = ps.tile([C, TW], f32, tag="ps")
                    nc.tensor.matmul(wp[:], Wz[:, :C], Wz[:],
                                     start=True, stop=True)

            warm(10)

            Xpv = Xpad[:].rearrange("p (r c) -> p r c", r=H + 2)

            # ---------------- CBL_Q: conv3x3 + batch stats ----------------
            # Conv output in BLOCK-MAJOR key order: chunk t covers block rows
            # n=2t,2t+1; column n*512 + m*64 + p*8 + q is pixel (8n+p, 8m+q).
            Zq = big.tile([C, NCC, TW], bf16)
            qstats = small.tile([C, 2 * NCC, 6], f32)
            for t in range(NCC):
                pq = ps.tile([C, TW], f32, tag="ps")
                for tap in range(9):
                    dh, dw = tap // 3 - 1, tap % 3 - 1
                    rhs = Xpv[:, t * 16 + 1 + dh: t * 16 + 17 + dh,
                              1 + dw: 65 + dw].rearrange(
                                  "c (n p) (m q) -> c n m p q", n=2, m=8)
                    nc.tensor.matmul(pq[:], Wq_s[:, tap, :], rhs,
                                     start=(tap == 0), stop=(tap == 8))
                nc.vector.bn_stats(qstats[:, 2 * t, :], pq[:, :512])
                nc.vector.bn_stats(qstats[:, 2 * t + 1, :], pq[:, 512:])
                nc.gpsimd.tensor_copy(Zq[:, t, :], pq[:])

            qmv = small.tile([C, 2], f32)
            nc.vector.bn_aggr(qmv[:], qstats[:])
            # partial sums for the cross-core stats:
            #   sums[:,0] = mean * 4096 ; sums[:,1] = (var + mean^2) * 4096
            sums = small.tile([C, 2], f32)
            nc.vector.tensor_scalar_mul(sums[:, 0:1], qmv[:, 0:1], float(HWPIX))
            m2 = small.tile([C, 1], f32)
            nc.vector.tensor_mul(m2[:], qmv[:, 0:1], qmv[:, 0:1])
            nc.vector.tensor_add(m2[:], m2[:], qmv[:, 1:2])
            nc.vector.tensor_scalar_mul(sums[:, 1:2], m2[:], float(HWPIX))

            grp = [list(range(N_CORES))]

            def gather8(name, src, width, warm_n=0):
                """AllGather a [C,width] fp32 tile -> [C,8,width] on-chip."""
                t_in = dram.tile([C, width], f32, tag=f"{name}_in")
                t_out = dram.tile([N_CORES * C, width], f32,
                                  addr_space="Shared", tag=f"{name}_out")
                nc.sync.dma_start(t_in[:], src[:])
                nc.gpsimd.collective_compute(
                    "AllGather", OP.bypass, replica_groups=grp,
                    ins=[t_in.opt()], outs=[t_out.opt()])
                if warm_n:
                    warm(warm_n)
                g = small.tile([C, 8, width], f32, tag=f"{name}_g")
                nc.sync.dma_start(
                    g[:], t_out[:].rearrange("(r p) s -> p r s", r=N_CORES))
                return g

            def bn_affine(gst8, tot, gamma, beta=None):
                """gst8: [C,8,2] gathered (sum, sumsq) partials. Returns
                (a, b): a = gamma*rsqrt(var+eps), b = beta - a*mean (b=None
                when beta is None -- the shift cancels downstream)."""
                gsum = small.tile([C, 2], f32, tag="gsum")
                nc.vector.tensor_reduce(
                    gsum[:], gst8[:].rearrange("c r s -> c s r"),
                    axis=AX.X, op=OP.add)
                sc = small.tile([C, 2], f32, tag="scmom")
                nc.vector.tensor_scalar_mul(sc[:], gsum[:], 1.0 / tot)
                negvar = small.tile([C, 1], f32, tag="negvar")
                nc.vector.scalar_tensor_tensor(negvar[:], sc[:, 0:1],
                                               sc[:, 0:1], sc[:, 1:2],
                                               op0=OP.mult, op1=OP.subtract)
                # rsqrt(var+eps) = exp(-0.5*ln(var+eps)); ln/exp share one
                # ACT table set, so no LoadActFuncSet in this chain
                lnv = small.tile([C, 1], f32, tag="lnv")
                nc.scalar.activation(lnv[:], negvar[:], AF.Ln,
                                     scale=-1.0, bias=eps_t[:])
                rstd = small.tile([C, 1], f32, tag="rstd")
                nc.scalar.activation(rstd[:], lnv[:], AF.Exp, scale=-0.5)
                a = small.tile([C, 1], f32, tag="acoef")
                nc.vector.tensor_mul(a[:], rstd[:], gamma)
                if beta is None:
                    return a, None
                b = small.tile([C, 1], f32, tag="bcoef")
                nc.vector.tensor_scalar(b[:], sc[:, 0:1], a[:], -1.0,
                                        op0=OP.mult, op1=OP.mult)
                nc.vector.tensor_add(b[:], b[:], beta)
                return a, b

            gstq = gather8("stq", sums, 2, warm_n=56)
            aq, bq = bn_affine(gstq, float(HWPIX * N_CORES),
                               V[:, 0:1], V[:, 1:2])

            # q = leaky(aq*z + bq); Zq is already block-major. Chunk 0 is
            # emitted here; chunks 1-3 are interleaved into the main loop
            # (issued a few iterations before their keys are needed) so the
            # in-order ACT queue never blocks exp_0 behind them.
            Qc = big.tile([C, HWPIX], bf16)
            Qv = Qc[:].rearrange("p (t f) -> p t f", f=TW)

            def affine_chunk(t):
                tmp = tmp2p.tile([C, TW], f32, tag="tmp2")
                nc.scalar.activation(tmp[:], Zq[:, t, :], AF.Identity,
                                     scale=aq[:], bias=bq[:])
                nc.vector.scalar_tensor_tensor(Qv[:, t, :], tmp[:], ALPHA,
                                               tmp[:], op0=OP.mult,
                                               op1=OP.max)

            affine_chunk(0)

            # ---------------- attention main loop (software-pipelined) ----
            z1 = big.tile([C, NQT, TW], bf16)
            qs1 = small.tile([C, 2 * NQT, 6], f32)
            paggs = [None] * NQT
            Es = [None] * (NIT + 2)
            with nc.allow_low_precision(reason="attn weights in bf16"):
                for i in range(NIT + 2):
                    qt, kt = i // NKT, i % NKT
                    if qt == 0 and kt in (5, 13, 21):
                        affine_chunk(kt // 8 + 1)
                    # stage 0: scores (+ per-qt Conv_K group start)
                    if i < NIT:
                        if kt == 0 and qt == 0:
                            pagg0 = psA.tile([C, TW], f32, tag="agg")
                            paggs[0] = pagg0
                            nc.tensor.matmul(pagg0[:], Wk_s[:],
                                             Xq[:, :TW],
                                             start=True, stop=False)
                        if kt == 2 and qt + 1 < NQT:
                            # issued just after agg[qt,31] so the single
                            # psA slot never deadlocks the in-order PE queue
                            pagg_n = psA.tile([C, TW], f32, tag="agg")
                            paggs[qt + 1] = pagg_n
                            nc.tensor.matmul(
                                pagg_n[:], Wk_s[:],
                                Xq[:, (qt + 1) * TW:(qt + 2) * TW],
                                start=True, stop=False)
                        psS = ps.tile([C, TW], f32, tag="ps")
                        nc.tensor.matmul(psS[:],
                                         Qc[:, kt * 128:(kt + 1) * 128],
                                         Xq[:, qt * TW:(qt + 1) * TW],
                                         start=True, stop=True)
                        Ei = work.tile([C, TW], bf16, tag="E")
                        Es[i] = Ei
                        nc.scalar.activation(Ei[:], psS[:], AF.Exp,
                                             scale=1.0 / RF)
                    # stage 1: block-sum matmul + divide (A = E/D in place)
                    if 1 <= i < NIT + 1:
                        j = i - 1
                        psD = ps.tile([C, TW], f32, tag="ps")
                        nc.tensor.matmul(psD[:], Bb[:], Es[j][:],
                                         start=True, stop=True)
                        eng = nc.gpsimd if (POOL_EVERY and
                                            j % POOL_EVERY == POOL_EVERY - 1) \
                            else nc.vector
                        eng.tensor_tensor(Es[j][:], Es[j][:], psD[:],
                                          op=OP.divide)
                    # stage 2: weighted-sum accumulate
                    if i >= 2:
                        j = i - 2
                        jqt, jkt = j // NKT, j % NKT
                        nc.tensor.matmul(paggs[jqt][:], Xnat[:, jkt, :],
                                         Es[j][:], start=False,
                                         stop=(jkt == NKT - 1))
                        Es[j] = None
                        if jkt == NKT - 1:
                            nc.vector.bn_stats(qs1[:, 2 * jqt, :],
                                               paggs[jqt][:, :512])
                            nc.vector.bn_stats(qs1[:, 2 * jqt + 1, :],
                                               paggs[jqt][:, 512:])
                            nc.gpsimd.tensor_copy(z1[:, jqt, :],
                                                  paggs[jqt][:])
                            paggs[jqt] = None

            # ---------------- epilogue (sharded) ----------------
            # partial BN_1 sums for this core's 2048 pixels
            mv1 = small.tile([C, 2], f32)
            nc.vector.bn_aggr(mv1[:], qs1[:])
            sums1 = small.tile([C, 2], f32)
            nc.vector.tensor_scalar_mul(sums1[:, 0:1], mv1[:, 0:1], float(QSH))
            m21 = small.tile([C, 1], f32)
            nc.vector.tensor_mul(m21[:], mv1[:, 0:1], mv1[:, 0:1])
            nc.vector.tensor_add(m21[:], m21[:], mv1[:, 1:2])
            nc.vector.tensor_scalar_mul(sums1[:, 1:2], m21[:], float(QSH))

            gst1 = gather8("st1", sums1, 2)
            # only a1 = g1*rsqrt(var+eps) matters: the spatial softmax is
            # invariant to the BN_1 shift (and to bt1)
            a1, _b1 = bn_affine(gst1, float(B * HWPIX), V[:, 2:3])

            # E1 = exp(a1 * z1) in one ACT pass; accum gives the shard sum
            E1 = big.tile([C, NQT, TW], bf16)
            esum = small.tile([C, 1], f32)
            nc.scalar.activation(E1[:].rearrange("p a b -> p (a b)"),
                                 z1[:].rearrange("p a b -> p (a b)"),
                                 AF.Exp, scale=a1[:], accum_out=esum[:])

            ges = gather8("es", esum, 1)
            # this core's spatial-softmax denominator: the two partials of
            # its own batch pair, selected by the per-core one-hot mask
            gsel = small.tile([C, 8], f32)
            nc.vector.tensor_mul(gsel[:], ges[:, :, 0], Msk[:])
            den = small.tile([C, 1], f32)
            nc.vector.tensor_reduce(den[:], gsel[:], axis=AX.X, op=OP.add)
            rb = small.tile([C, 1], f32)
            nc.vector.reciprocal(rb[:], den[:])

            # y = E1 * rb (in place), then CBL_O conv + batch stats
            zO = big.tile([C, NQT, TW], bf16)
            stO = small.tile([C, 2 * NQT, 6], f32)
            for t in range(NQT):
                with nc.allow_low_precision(reason="softmax weights bf16"):
                    nc.vector.tensor_scalar_mul(E1[:, t, :], E1[:, t, :],
                                                rb[:])
                pzo = ps.tile([C, TW], f32, tag="ps")
                nc.tensor.matmul(pzo[:], Wo_s[:], E1[:, t, :],
                                 start=True, stop=True)
                nc.vector.bn_stats(stO[:, 2 * t, :], pzo[:, :512])
                nc.vector.bn_stats(stO[:, 2 * t + 1, :], pzo[:, 512:])
                nc.gpsimd.tensor_copy(zO[:, t, :], pzo[:])

            mvO = small.tile([C, 2], f32)
            nc.vector.bn_aggr(mvO[:], stO[:])
            sumsO = small.tile([C, 2], f32)
            nc.vector.tensor_scalar_mul(sumsO[:, 0:1], mvO[:, 0:1], float(QSH))
            m2O = small.tile([C, 1], f32)
            nc.vector.tensor_mul(m2O[:], mvO[:, 0:1], mvO[:, 0:1])
            nc.vector.tensor_add(m2O[:], m2O[:], mvO[:, 1:2])
            nc.vector.tensor_scalar_mul(sumsO[:, 1:2], m2O[:], float(QSH))

            gstO = gather8("stO", sumsO, 2)
            aO, bO = bn_affine(gstO, float(B * HWPIX), V[:, 4:5], V[:, 5:6])

            OUT = big.tile([C, NQT, TW], f32)
            for t in range(NQT):
                tmp = tmp2p.tile([C, TW], f32, tag="tmp2")
                nc.scalar.activation(tmp[:], zO[:, t, :], AF.Identity,
                                     scale=aO[:], bias=bO[:])
                nc.vector.scalar_tensor_tensor(OUT[:, t, :], tmp[:], ALPHA,
                                               tmp[:], op0=OP.mult,
                                               op1=OP.max)
                eng = nc.sync if t % 2 == 0 else nc.scalar
                eng.dma_start(d_outT[:, t * TW:(t + 1) * TW], OUT[:, t, :])

    nc.compile()
    return nc


def _get_runner():
    if "runner" in _CACHE:
        return _CACHE["runner"]
    import jax
    import numpy as np
    from jax.sharding import Mesh, PartitionSpec
    from jax.experimental.shard_map import shard_map
    from concourse import mybir
    from concourse.bass2jax import (_bass_exec_p, install_neuronx_cc_hook,
                                    partition_id_tensor)

    nc = _build_program()
    install_neuronx_cc_hook()

    in_names, out_names, out_avals, zero_outs = [], [], [], []
    partition_name = nc.partition_id_tensor.name if nc.partition_id_tensor else None
    for alloc in nc.m.functions[0].allocations:
        if not isinstance(alloc, mybir.MemoryLocationSet):
            continue
        name = alloc.memorylocations[0].name
        if alloc.kind == "ExternalInput":
            if name != partition_name:
                in_names.append(name)
        elif alloc.kind == "ExternalOutput":
            shape = tuple(alloc.tensor_shape)
            dtype = mybir.dt.np(alloc.dtype)
            out_names.append(name)
            out_avals.append(jax.core.ShapedArray(shape, dtype))
            zero_outs.append(np.zeros(shape, dtype))
    n_params = len(in_names)
    n_outs = len(out_avals)
    all_in_names = list(in_names) + list(out_names)
    if partition_name is not None:
        all_in_names.append(partition_name)

    def _body(*args):
        operands = list(args)
        if partition_name is not None:
            operands.append(partition_id_tensor())
        outs = _bass_exec_p.bind(
            *operands,
            out_avals=tuple(out_avals),
            in_names=tuple(all_in_names),
            out_names=tuple(out_names),
            lowering_input_output_aliases=(),
            sim_require_finite=True,
            sim_require_nnan=True,
            nc=nc,
        )
        return tuple(outs)

    donate = tuple(range(n_params, n_params + n_outs))
    try:
        devices = jax.devices("axon")[:N_CORES]
    except RuntimeError:
        devices = jax.devices()[:N_CORES]
    mesh = Mesh(np.asarray(devices), ("core",))
    in_specs = (PartitionSpec("core"),) * (n_params + n_outs)
    out_specs = (PartitionSpec("core"),) * n_outs
    sharded = jax.jit(
        shard_map(_body, mesh=mesh, in_specs=in_specs, out_specs=out_specs,
                  check_rep=False),
        donate_argnums=donate, keep_unused=True)

    def run(in_maps):
        per_core = [[np.asarray(m[name]) for name in in_names] for m in in_maps]
        concat_in = [np.concatenate([per_core[c][i] for c in range(N_CORES)],
                                    axis=0) for i in range(n_params)]
        concat_zeros = [np.zeros((N_CORES * z.shape[0], *z.shape[1:]), z.dtype)
                        for z in zero_outs]
        out_arrs = jax.block_until_ready(sharded(*concat_in, *concat_zeros))
        return [
            {name: np.asarray(out_arrs[i]).reshape(N_CORES, *out_avals[i].shape)[c]
             for i, name in enumerate(out_names)}
            for c in range(N_CORES)
        ]

    _CACHE["runner"] = run
    return run


def _make_blockmap():
    bm = np.zeros((C, C), np.float32)
    idx = np.arange(C)
    bm[(idx[:, None] // 64) == (idx[None, :] // 64)] = 1.0
    return bm


def kernel(x, Wq, bq, gq, btq, Wk, bk, g1, bt1, Wo, bo, go, bto):
    """Full inputs -> full output. Conv biases cancel inside training-mode
    BN (the mean subtraction removes any per-channel constant), so bq/bk/bo
    never enter the device program."""
    import ml_dtypes
    bf16 = ml_dtypes.bfloat16

    x = np.asarray(x, np.float32)
    run = _get_runner()

    wq9 = np.ascontiguousarray(
        np.asarray(Wq, np.float32).reshape(9, C, C)).astype(bf16)
    wk = np.ascontiguousarray(
        np.asarray(Wk, np.float32).reshape(C, C)).astype(bf16)
    wo = np.ascontiguousarray(
        np.asarray(Wo, np.float32).reshape(C, C)).astype(bf16)
    vecs = np.ascontiguousarray(np.stack([
        np.asarray(v, np.float32) for v in (gq, btq, g1, bt1, go, bto)]))
    bm = _make_blockmap().astype(bf16)

    # block-major key permutation: index (n,m,p,q) -> pixel (8n+p, 8m+q)
    perm = np.arange(HWPIX).reshape(8, 8, 8, 8).transpose(0, 2, 1, 3).reshape(-1)

    in_maps = []
    for core in range(N_CORES):
        b, h = core // 2, core % 2
        xb = np.ascontiguousarray(x[b].reshape(HWPIX, C))
        xbT = xb.T  # [C, HWPIX]
        xqT = np.ascontiguousarray(xbT[:, h * QSH:(h + 1) * QSH]).astype(bf16)
        xpadT = np.zeros((C, H + 2, W + 2), np.float32)
        xpadT[:, 1:H + 1, 1:W + 1] = xbT.reshape(C, H, W)
        msk = np.zeros((8, C), np.float32)
        msk[2 * b] = 1.0
        msk[2 * b + 1] = 1.0
        in_maps.append({
            "xnat": np.ascontiguousarray(xb[perm]).astype(bf16),
            "xqT": xqT,
            "xpadT": np.ascontiguousarray(
                xpadT.reshape(C, PADN)).astype(bf16),
            "wq9": wq9, "wk": wk, "wo": wo, "vecs": vecs, "msk": msk,
            "bm": bm,
        })

    res = run(in_maps)
    full = np.empty((B, HWPIX, C), np.float32)
    for core in range(N_CORES):
        b, h = core // 2, core % 2
        full[b, h * QSH:(h + 1) * QSH, :] = res[core]["outT"].T
    return full.reshape(B, H, W, C)
